# revision 9
# baseline (speedup 1.0000x reference)
"""BilateralFilter (SqueezeSeg mc condensing-kernel gaussians) on 8 TRN2 cores.

Reference computes, for x: [16, 64, 512, 3] (B, Z, A, C=xyz):
    nbr   = 14 spatial neighbors of each pixel in a 3x5 window (zero-padded)
    diff2 = sum_c (x - nbr)^2                           [B, Z, A, 14]
    out   = exp(-diff2 / (2 * theta_r^2))               [B, Z, A, 14, 4]
with THETA_R = [0.015, 0.015, 0.01, 0.01] (only 2 distinct values).

Strategy (pure batch data-parallel, 2 batches per core):
  - partitions p = b*64 + z  (128), free dim = azimuth chunks of 256 (x2).
  - mirror symmetry: m_k(q) = |x(q) - x(q+off_k)|^2 for the 7 "negative"
    offsets k=0..6 gives the other 7 via diff2_{13-k}(q) = m_k(q - off_k);
    the z+1-partition read is materialized by an SBUF->SBUF partition-remap
    DMA (M_up), with the phantom z=64 row filled from s = |x|^2 by a
    stride-0 DMA (out-of-image neighbor => diff2 = |x(center)|^2).
  - ACT computes exp with the free scale immediate; each exp is written to
    both classes of its theta pair via a stride-0 input axis.
  - output staged in SBUF exactly in DRAM layout [a, k, c] so the store DMA
    is fully contiguous per partition (57 KB/partition runs).
"""

import numpy as np

import concourse.bass as bass
import concourse.tile as tile
from concourse import bacc, mybir
from concourse.bass_utils import run_bass_kernel_spmd

N_CORES = 8
B, Z, A, C = 16, 64, 512, 3
K, NCLS = 14, 4
LB = B // N_CORES            # local batches per core = 2
P = LB * Z                   # 128 partitions
AC = 128                     # azimuth chunk
BUFS = 3                     # tile pool buffers
XDN_DRAM = True              # load x_dn straight from DRAM (parallel) vs sb2sb
F32 = mybir.dt.float32

# exp scales: -1 / (2 * theta^2), theta pairs (0.015, 0.01), f32 semantics
_t0 = np.float32(0.015)
_t1 = np.float32(0.01)
SC0 = -float(1.0 / np.float32(np.float32(2.0) * _t0 * _t0))
SC1 = -float(1.0 / np.float32(np.float32(2.0) * _t1 * _t1))

# DRAM strides (elements) of out [LB, Z, A, K, NCLS]
O_A = K * NCLS               # 56
O_Z = A * O_A                # 28672
O_B = Z * O_Z                # 1835008
X_Z = A * C                  # 1536
X_B = Z * X_Z


def _ap(t, poff, pcnt, foff, pairs, pstep=1):
    """AP on tile t: partitions [poff, poff+pcnt) (stride pstep rows), free
    `pairs` ([step, count] in elements) based at element foff."""
    row = t.ap[0][0]
    return bass.AP(tensor=t.tensor, offset=t.offset + poff * row + foff,
                   ap=[[pstep * row, pcnt]] + [list(p) for p in pairs])


_SQDIFF = None


def _get_sqdiff():
    """Register a runtime custom DVE op: out = (in0 - in1)^2 (fp32, one
    instruction instead of subtract + multiply)."""
    global _SQDIFF
    if _SQDIFF is not None:
        return _SQDIFF
    from concourse import dve_ops
    from concourse.dve_spec import Spec, Src0, Src1, sq, lower, _has_src1
    from concourse.dve_uop import DveOpSpec

    name = "SQDIFF_BILAT_ANT"
    if name not in dve_ops._SUB_OPCODE_FOR_NAME:
        spec = Spec(
            body=sq(Src0 - Src1),
            reference=lambda in0, in1, c0, c1, c2:
                (in0.astype(np.float32) - in1.astype(np.float32)) ** 2)
        row = 1 + len(dve_ops.OPS)
        assert row < 0x20
        shas = {}
        for ver in ("v3",):
            tmp = DveOpSpec(name=name, opcode=row, uops=lower(spec, ver=ver),
                            rd1_en=_has_src1(spec))
            shas[ver] = tmp.sha(ver)
        op = dve_ops.DveOp(name, spec, subdim=False, uops_sha=shas)
        dve_ops.OPS.append(op)
        dve_ops.CUSTOM_DVE_SPECS[name] = spec
        dve_ops._SUB_OPCODE_FOR_NAME[name] = row
    else:
        op = next(o for o in dve_ops.OPS if o.name == name)
    _SQDIFF = op
    return op


def _build(ac=AC, bufs=BUFS, xdn_dram=XDN_DRAM):
    XW = ac + 8                  # x window (halo 4 each side)
    MW = ac + 4                  # m window (halo 2 each side)
    NCH = A // ac
    nc = bacc.Bacc("TRN2", target_bir_lowering=False, debug=False,
                   num_devices=N_CORES)
    x_h = nc.dram_tensor("x", [LB, Z, A, C], F32, kind="ExternalInput")
    o_h = nc.dram_tensor("out", [LB, Z, A, K, NCLS], F32, kind="ExternalOutput")
    x_ap, o_ap = x_h.ap(), o_h.ap()

    with tile.TileContext(nc) as tc:
        with tc.tile_pool(name="pool", bufs=bufs) as pool:
            for ci in range(NCH):
                a0 = ci * ac
                lo, hi = max(0, a0 - 4), min(A, a0 + ac + 4)
                c_lo = (lo - (a0 - 4)) * C          # first valid xt col
                c_hi = (hi - (a0 - 4)) * C

                # ---- load x window (zero halo at image borders) ----
                xt = pool.tile([P, XW * C], F32, name="xt")
                if c_lo > 0:
                    nc.gpsimd.memset(_ap(xt, 0, P, 0, [[1, c_lo]]), 0.0)
                if c_hi < XW * C:
                    nc.gpsimd.memset(
                        _ap(xt, 0, P, c_hi, [[1, XW * C - c_hi]]), 0.0)
                for b in range(LB):
                    nc.sync.dma_start(
                        _ap(xt, b * Z, Z, c_lo, [[C, hi - lo], [1, C]]),
                        bass.AP(tensor=x_ap.tensor, offset=b * X_B + lo * C,
                                ap=[[X_Z, Z], [C, hi - lo], [1, C]]))

                # ---- x_dn[p] = x at (z-1) (zeros at z=0 rows) ----
                x_dn = pool.tile([P, XW * C], F32, name="x_dn")
                nc.gpsimd.memset(x_dn[:], 0.0)
                for b in range(LB):
                    if xdn_dram:
                        nc.sync.dma_start(
                            _ap(x_dn, b * Z + 1, Z - 1, c_lo,
                                [[C, hi - lo], [1, C]]),
                            bass.AP(tensor=x_ap.tensor,
                                    offset=b * X_B + lo * C,
                                    ap=[[X_Z, Z - 1], [C, hi - lo], [1, C]]))
                    else:
                        nc.sync.dma_start(
                            _ap(x_dn, b * Z + 1, Z - 1, 0, [[1, XW * C]]),
                            _ap(xt, b * Z, Z - 1, 0, [[1, XW * C]]))

                # ---- s = sum_c x^2 over the full x window ----
                sqx = pool.tile([P, XW * C], F32, name="sqx")
                nc.scalar.square(sqx[:], xt[:])
                st = pool.tile([P, XW], F32, name="st")
                nc.vector.tensor_reduce(
                    st[:], _ap(sqx, 0, P, 0, [[C, XW], [1, C]]),
                    axis=mybir.AxisListType.X, op=mybir.AluOpType.add)

                # ---- m_k maps: M[p, k*MW + ar] over a-window [a0-2, a0+258)
                # k=0..4: dz=-1, da=k-2 ; k=5,6: dz=0, da=k-7
                # d2 = (x - x_nbr)^2 in one fused custom op per k
                sqdiff = _get_sqdiff()
                M = pool.tile([P, 7 * MW], F32, name="M")
                dt5 = pool.tile([P, 5 * MW * C], F32, name="dt5")
                for k in range(5):
                    nc.vector._custom_dve(
                        sqdiff,
                        out=_ap(dt5, 0, P, k * MW * C, [[C, MW], [1, C]]),
                        in0=_ap(xt, 0, P, 2 * C, [[C, MW], [1, C]]),
                        in1=_ap(x_dn, 0, P, k * C, [[C, MW], [1, C]]))
                nc.vector.tensor_reduce(
                    _ap(M, 0, P, 0, [[1, 5 * MW]]),
                    _ap(dt5, 0, P, 0, [[C, 5 * MW], [1, C]]),
                    axis=mybir.AxisListType.X, op=mybir.AluOpType.add)

                dt2 = pool.tile([P, 2 * MW * C], F32, name="dt2")
                for k in (5, 6):
                    nc.vector._custom_dve(
                        sqdiff,
                        out=_ap(dt2, 0, P, (k - 5) * MW * C, [[C, MW], [1, C]]),
                        in0=_ap(xt, 0, P, 2 * C, [[C, MW], [1, C]]),
                        in1=_ap(xt, 0, P, (k - 5) * C, [[C, MW], [1, C]]))
                nc.vector.tensor_reduce(
                    _ap(M, 0, P, 5 * MW, [[1, 2 * MW]]),
                    _ap(dt2, 0, P, 0, [[C, 2 * MW], [1, C]]),
                    axis=mybir.AxisListType.X, op=mybir.AluOpType.add)

                # ---- M_up[p] = M[p+1] for k=0..4 cols; phantom z=64 rows
                # ({63,127}) = s(z=63 row) with k-dependent a-shift ----
                M_up = pool.tile([P, 5 * MW], F32, name="M_up")
                for b in range(LB):
                    nc.sync.dma_start(
                        _ap(M_up, b * Z, Z - 1, 0, [[1, 5 * MW]]),
                        _ap(M, b * Z + 1, Z - 1, 0, [[1, 5 * MW]]))
                # phantom: M_up[{63,127}, k*MW + ar] = st[{63,127}, ar + k]
                nc.sync.dma_start(
                    _ap(M_up, Z - 1, 2, 0, [[MW, 5], [1, MW]], pstep=Z),
                    _ap(st, Z - 1, 2, 0, [[1, 5], [1, MW]], pstep=Z))

                # ---- exps into O staging [p, ar*56 + k*4 + c] ----
                O = pool.tile([P, ac * O_A], F32, name="O")
                for th, sc in ((0, SC0), (1, SC1)):
                    co = 2 * th
                    # direct k=0..6: in M[p, k*MW + ar + 2]
                    nc.scalar.activation(
                        _ap(O, 0, P, co, [[4, 7], [O_A, ac], [1, 2]]),
                        _ap(M, 0, P, 2, [[MW, 7], [1, ac], [0, 2]]),
                        mybir.ActivationFunctionType.Exp, scale=sc)
                    # a-mirrors k'=7,8 <- k=6,5: col = k*MW + ar + (9-k)
                    nc.scalar.activation(
                        _ap(O, 0, P, 28 + co, [[4, 2], [O_A, ac], [1, 2]]),
                        _ap(M, 0, P, 6 * MW + 3, [[-(MW - 1), 2], [1, ac], [0, 2]]),
                        mybir.ActivationFunctionType.Exp, scale=sc)
                    # dz-mirrors k'=9..13 <- k=4..0: M_up[p, k*MW + ar + 4 - k]
                    nc.scalar.activation(
                        _ap(O, 0, P, 36 + co, [[4, 5], [O_A, ac], [1, 2]]),
                        _ap(M_up, 0, P, 4 * (MW - 1) + 4,
                            [[-(MW - 1), 5], [1, ac], [0, 2]]),
                        mybir.ActivationFunctionType.Exp, scale=sc)

                # ---- store ----
                for b in range(LB):
                    nc.sync.dma_start(
                        bass.AP(tensor=o_ap.tensor,
                                offset=b * O_B + a0 * O_A,
                                ap=[[O_Z, Z], [1, ac * O_A]]),
                        _ap(O, b * Z, Z, 0, [[1, ac * O_A]]))

    nc.compile()
    return nc


class _Runner:
    """Compile once; reuse the jitted sharded executable across calls.

    Mirrors bass2jax.run_bass_via_pjrt's multi-core path, but without
    donated output buffers (the kernel writes every output element, so the
    zero "output operands" are passed once from device-resident buffers and
    reused)."""

    def __init__(self):
        import jax
        from jax.sharding import Mesh, PartitionSpec, NamedSharding
        try:
            from jax.experimental.shard_map import shard_map
        except ImportError:
            from jax.shard_map import shard_map  # newer jax
        from concourse import bass2jax

        bass2jax.install_neuronx_cc_hook()
        nc = _build()
        self.nc = nc

        partition_name = (nc.partition_id_tensor.name
                          if nc.partition_id_tensor else None)
        in_names, out_names, out_avals = [], [], []
        for alloc in nc.m.functions[0].allocations:
            if not isinstance(alloc, mybir.MemoryLocationSet):
                continue
            name = alloc.memorylocations[0].name
            if alloc.kind == "ExternalInput":
                if name != partition_name:
                    in_names.append(name)
            elif alloc.kind == "ExternalOutput":
                out_names.append(name)
                out_avals.append(jax.core.ShapedArray(
                    tuple(alloc.tensor_shape), mybir.dt.np(alloc.dtype)))
        assert in_names == ["x"] and out_names == ["out"], (in_names, out_names)
        all_in_names = in_names + out_names
        if partition_name is not None:
            all_in_names = all_in_names + [partition_name]

        def _body(*args):
            operands = list(args)
            if partition_name is not None:
                operands.append(bass2jax.partition_id_tensor())
            return tuple(bass2jax._bass_exec_p.bind(
                *operands,
                out_avals=tuple(out_avals),
                in_names=tuple(all_in_names),
                out_names=tuple(out_names),
                lowering_input_output_aliases=(),
                sim_require_finite=True,
                sim_require_nnan=True,
                nc=nc,
            ))

        devices = jax.devices()[:N_CORES]
        assert len(devices) == N_CORES
        self.mesh = Mesh(np.asarray(devices), ("core",))
        spec = PartitionSpec("core")
        self.sharding = NamedSharding(self.mesh, spec)
        self.jitted = jax.jit(shard_map(
            _body, mesh=self.mesh, in_specs=(spec, spec), out_specs=(spec,),
            check_rep=False))
        # device-resident dummy output operand, created once
        self.zeros_dev = jax.device_put(
            np.zeros((N_CORES * LB, Z, A, K, NCLS), np.float32), self.sharding)
        self._jax = jax

    def put(self, x: np.ndarray):
        return self._jax.device_put(
            np.ascontiguousarray(np.asarray(x, np.float32)), self.sharding)

    def run_dev(self, x_dev):
        """Execute; returns device array (not fetched)."""
        return self.jitted(x_dev, self.zeros_dev)[0]

    def __call__(self, x: np.ndarray) -> np.ndarray:
        return np.asarray(self.run_dev(self.put(x)))


_RUNNER = None


def _get_runner():
    global _RUNNER
    if _RUNNER is None:
        _RUNNER = _Runner()
    return _RUNNER


def kernel(x: np.ndarray) -> np.ndarray:
    x = np.asarray(x, dtype=np.float32)
    assert x.shape == (B, Z, A, C), x.shape
    try:
        return _get_runner()(x)
    except Exception:
        # fallback: reference-quality but slower dispatch path
        nc = _build()
        in_maps = [{"x": np.ascontiguousarray(x[i * LB:(i + 1) * LB])}
                   for i in range(N_CORES)]
        res = run_bass_kernel_spmd(nc, in_maps, list(range(N_CORES)))
        return np.concatenate(
            [res.results[i]["out"] for i in range(N_CORES)], axis=0)


# revision 14
# speedup vs baseline: 1.0302x; 1.0302x over previous
"""BilateralFilter (SqueezeSeg mc condensing-kernel gaussians) on 8 TRN2 cores.

Reference computes, for x: [16, 64, 512, 3] (B, Z, A, C=xyz):
    nbr   = 14 spatial neighbors of each pixel in a 3x5 window (zero-padded)
    diff2 = sum_c (x - nbr)^2                           [B, Z, A, 14]
    out   = exp(-diff2 / (2 * theta_r^2))               [B, Z, A, 14, 4]
with THETA_R = [0.015, 0.015, 0.01, 0.01] (only 2 distinct values).

Strategy (pure batch data-parallel, 2 batches per core):
  - partitions p = b*64 + z  (128), free dim = azimuth chunks of 256 (x2).
  - mirror symmetry: m_k(q) = |x(q) - x(q+off_k)|^2 for the 7 "negative"
    offsets k=0..6 gives the other 7 via diff2_{13-k}(q) = m_k(q - off_k);
    the z+1-partition read is materialized by an SBUF->SBUF partition-remap
    DMA (M_up), with the phantom z=64 row filled from s = |x|^2 by a
    stride-0 DMA (out-of-image neighbor => diff2 = |x(center)|^2).
  - ACT computes exp with the free scale immediate; each exp is written to
    both classes of its theta pair via a stride-0 input axis.
  - output staged in SBUF exactly in DRAM layout [a, k, c] so the store DMA
    is fully contiguous per partition (57 KB/partition runs).
"""

import numpy as np

import concourse.bass as bass
import concourse.tile as tile
from concourse import bacc, mybir
from concourse.bass_utils import run_bass_kernel_spmd

N_CORES = 8
B, Z, A, C = 16, 64, 512, 3
K, NCLS = 14, 4
LB = B // N_CORES            # local batches per core = 2
P = LB * Z                   # 128 partitions
AC = 128                     # azimuth chunk
BUFS = 3                     # tile pool buffers
XDN_DRAM = True              # load x_dn straight from DRAM (parallel) vs sb2sb
F32 = mybir.dt.float32

# exp scales: -1 / (2 * theta^2), theta pairs (0.015, 0.01), f32 semantics
_t0 = np.float32(0.015)
_t1 = np.float32(0.01)
SC0 = -float(1.0 / np.float32(np.float32(2.0) * _t0 * _t0))
SC1 = -float(1.0 / np.float32(np.float32(2.0) * _t1 * _t1))

# DRAM strides (elements) of out [LB, Z, A, K, NCLS]
O_A = K * NCLS               # 56
O_Z = A * O_A                # 28672
O_B = Z * O_Z                # 1835008
X_Z = A * C                  # 1536
X_B = Z * X_Z


def _ap(t, poff, pcnt, foff, pairs, pstep=1):
    """AP on tile t: partitions [poff, poff+pcnt) (stride pstep rows), free
    `pairs` ([step, count] in elements) based at element foff."""
    row = t.ap[0][0]
    return bass.AP(tensor=t.tensor, offset=t.offset + poff * row + foff,
                   ap=[[pstep * row, pcnt]] + [list(p) for p in pairs])


_SQDIFF = None


def _get_sqdiff():
    """Register a runtime custom DVE op: out = (in0 - in1)^2 (fp32, one
    instruction instead of subtract + multiply)."""
    global _SQDIFF
    if _SQDIFF is not None:
        return _SQDIFF
    from concourse import dve_ops
    from concourse.dve_spec import Spec, Src0, Src1, sq, lower, _has_src1
    from concourse.dve_uop import DveOpSpec

    name = "SQDIFF_BILAT_ANT"
    if name not in dve_ops._SUB_OPCODE_FOR_NAME:
        spec = Spec(
            body=sq(Src0 - Src1),
            reference=lambda in0, in1, c0, c1, c2:
                (in0.astype(np.float32) - in1.astype(np.float32)) ** 2)
        row = 1 + len(dve_ops.OPS)
        assert row < 0x20
        shas = {}
        for ver in ("v3",):
            tmp = DveOpSpec(name=name, opcode=row, uops=lower(spec, ver=ver),
                            rd1_en=_has_src1(spec))
            shas[ver] = tmp.sha(ver)
        op = dve_ops.DveOp(name, spec, subdim=False, uops_sha=shas)
        dve_ops.OPS.append(op)
        dve_ops.CUSTOM_DVE_SPECS[name] = spec
        dve_ops._SUB_OPCODE_FOR_NAME[name] = row
    else:
        op = next(o for o in dve_ops.OPS if o.name == name)
    _SQDIFF = op
    return op


def _build(ac=AC, bufs=BUFS, xdn_dram=XDN_DRAM, reps=1):
    XW = ac + 8                  # x window (halo 4 each side)
    MW = ac + 4                  # m window (halo 2 each side)
    NCH = A // ac
    nc = bacc.Bacc("TRN2", target_bir_lowering=False, debug=False,
                   num_devices=N_CORES)
    x_h = nc.dram_tensor("x", [LB, Z, A, C], F32, kind="ExternalInput")
    o_h = nc.dram_tensor("out", [LB, Z, A, K, NCLS], F32, kind="ExternalOutput")
    x_ap, o_ap = x_h.ap(), o_h.ap()

    with tile.TileContext(nc) as tc:
        with tc.tile_pool(name="pool", bufs=bufs) as pool:
            for ci in range(NCH * reps):
                a0 = (ci % NCH) * ac
                lo, hi = max(0, a0 - 4), min(A, a0 + ac + 4)
                c_lo = (lo - (a0 - 4)) * C          # first valid xt col
                c_hi = (hi - (a0 - 4)) * C

                # ---- load x window (zero halo at image borders) ----
                # (b, z) rows are contiguous in DRAM: one 128-partition DMA
                xt = pool.tile([P, XW * C], F32, name="xt")
                if c_lo > 0:
                    nc.gpsimd.memset(_ap(xt, 0, P, 0, [[1, c_lo]]), 0.0)
                if c_hi < XW * C:
                    nc.gpsimd.memset(
                        _ap(xt, 0, P, c_hi, [[1, XW * C - c_hi]]), 0.0)
                nc.sync.dma_start(
                    _ap(xt, 0, P, c_lo, [[C, hi - lo], [1, C]]),
                    bass.AP(tensor=x_ap.tensor, offset=lo * C,
                            ap=[[X_Z, P], [C, hi - lo], [1, C]]))

                # ---- x_dn[p] = x at (z-1) (zeros at z=0 rows) ----
                # one DMA for rows 1..127 (row 64 gets cross-batch garbage,
                # re-zeroed after), plus halo/row-0 zeros from the memset
                x_dn = pool.tile([P, XW * C], F32, name="x_dn")
                nc.gpsimd.memset(x_dn[:], 0.0)
                nc.sync.dma_start(
                    _ap(x_dn, 1, P - 1, c_lo, [[C, hi - lo], [1, C]]),
                    bass.AP(tensor=x_ap.tensor, offset=lo * C,
                            ap=[[X_Z, P - 1], [C, hi - lo], [1, C]]))
                nc.gpsimd.memset(_ap(x_dn, Z, 1, 0, [[1, XW * C]]), 0.0)

                # ---- s = sum_c x^2 over the full x window ----
                sqx = pool.tile([P, XW * C], F32, name="sqx")
                nc.scalar.square(sqx[:], xt[:])
                st = pool.tile([P, XW], F32, name="st")
                nc.vector.tensor_reduce(
                    st[:], _ap(sqx, 0, P, 0, [[C, XW], [1, C]]),
                    axis=mybir.AxisListType.X, op=mybir.AluOpType.add)

                # ---- m_k maps: M[p, k*MW + ar] over a-window [a0-2, a0+258)
                # k=0..4: dz=-1, da=k-2 ; k=5,6: dz=0, da=k-7
                # d2 = (x - x_nbr)^2 in one fused custom op per k
                sqdiff = _get_sqdiff()
                M = pool.tile([P, 7 * MW], F32, name="M")
                dt5 = pool.tile([P, 5 * MW * C], F32, name="dt5")
                for k in range(5):
                    nc.vector._custom_dve(
                        sqdiff,
                        out=_ap(dt5, 0, P, k * MW * C, [[C, MW], [1, C]]),
                        in0=_ap(xt, 0, P, 2 * C, [[C, MW], [1, C]]),
                        in1=_ap(x_dn, 0, P, k * C, [[C, MW], [1, C]]))
                nc.vector.tensor_reduce(
                    _ap(M, 0, P, 0, [[1, 5 * MW]]),
                    _ap(dt5, 0, P, 0, [[C, 5 * MW], [1, C]]),
                    axis=mybir.AxisListType.X, op=mybir.AluOpType.add)

                dt2 = pool.tile([P, 2 * MW * C], F32, name="dt2")
                for k in (5, 6):
                    nc.vector._custom_dve(
                        sqdiff,
                        out=_ap(dt2, 0, P, (k - 5) * MW * C, [[C, MW], [1, C]]),
                        in0=_ap(xt, 0, P, 2 * C, [[C, MW], [1, C]]),
                        in1=_ap(xt, 0, P, (k - 5) * C, [[C, MW], [1, C]]))
                nc.vector.tensor_reduce(
                    _ap(M, 0, P, 5 * MW, [[1, 2 * MW]]),
                    _ap(dt2, 0, P, 0, [[C, 2 * MW], [1, C]]),
                    axis=mybir.AxisListType.X, op=mybir.AluOpType.add)

                # ---- M_up[p] = M[p+1] for k=0..4 cols; phantom z=64 rows
                # ({63,127}) = s(z=63 row) with k-dependent a-shift ----
                M_up = pool.tile([P, 5 * MW], F32, name="M_up")
                # rows 0..126 <- M[1..127]; row 63 gets cross-batch garbage,
                # overwritten by the phantom DMA below (order = WAW dep)
                nc.sync.dma_start(
                    _ap(M_up, 0, P - 1, 0, [[1, 5 * MW]]),
                    _ap(M, 1, P - 1, 0, [[1, 5 * MW]]))
                # phantom: M_up[{63,127}, k*MW + ar] = st[{63,127}, ar + k]
                nc.sync.dma_start(
                    _ap(M_up, Z - 1, 2, 0, [[MW, 5], [1, MW]], pstep=Z),
                    _ap(st, Z - 1, 2, 0, [[1, 5], [1, MW]], pstep=Z))

                # ---- exps into O staging [p, ar*56 + k*4 + c] ----
                O = pool.tile([P, ac * O_A], F32, name="O")
                for th, sc in ((0, SC0), (1, SC1)):
                    co = 2 * th
                    # direct k=0..6: in M[p, k*MW + ar + 2]
                    nc.scalar.activation(
                        _ap(O, 0, P, co, [[4, 7], [O_A, ac], [1, 2]]),
                        _ap(M, 0, P, 2, [[MW, 7], [1, ac], [0, 2]]),
                        mybir.ActivationFunctionType.Exp, scale=sc)
                    # a-mirrors k'=7,8 <- k=6,5: col = k*MW + ar + (9-k)
                    nc.scalar.activation(
                        _ap(O, 0, P, 28 + co, [[4, 2], [O_A, ac], [1, 2]]),
                        _ap(M, 0, P, 6 * MW + 3, [[-(MW - 1), 2], [1, ac], [0, 2]]),
                        mybir.ActivationFunctionType.Exp, scale=sc)
                    # dz-mirrors k'=9..13 <- k=4..0: M_up[p, k*MW + ar + 4 - k]
                    nc.scalar.activation(
                        _ap(O, 0, P, 36 + co, [[4, 5], [O_A, ac], [1, 2]]),
                        _ap(M_up, 0, P, 4 * (MW - 1) + 4,
                            [[-(MW - 1), 5], [1, ac], [0, 2]]),
                        mybir.ActivationFunctionType.Exp, scale=sc)

                # ---- store: one contiguous 128-partition DMA ----
                nc.sync.dma_start(
                    bass.AP(tensor=o_ap.tensor, offset=a0 * O_A,
                            ap=[[O_Z, P], [1, ac * O_A]]),
                    _ap(O, 0, P, 0, [[1, ac * O_A]]))

    nc.compile()
    return nc


class _Runner:
    """Compile once; reuse the jitted sharded executable across calls.

    Mirrors bass2jax.run_bass_via_pjrt's multi-core path, but without
    donated output buffers (the kernel writes every output element, so the
    zero "output operands" are passed once from device-resident buffers and
    reused)."""

    def __init__(self):
        import jax
        from jax.sharding import Mesh, PartitionSpec, NamedSharding
        try:
            from jax.experimental.shard_map import shard_map
        except ImportError:
            from jax.shard_map import shard_map  # newer jax
        from concourse import bass2jax

        bass2jax.install_neuronx_cc_hook()
        nc = _build()
        self.nc = nc

        partition_name = (nc.partition_id_tensor.name
                          if nc.partition_id_tensor else None)
        in_names, out_names, out_avals = [], [], []
        for alloc in nc.m.functions[0].allocations:
            if not isinstance(alloc, mybir.MemoryLocationSet):
                continue
            name = alloc.memorylocations[0].name
            if alloc.kind == "ExternalInput":
                if name != partition_name:
                    in_names.append(name)
            elif alloc.kind == "ExternalOutput":
                out_names.append(name)
                out_avals.append(jax.core.ShapedArray(
                    tuple(alloc.tensor_shape), mybir.dt.np(alloc.dtype)))
        assert in_names == ["x"] and out_names == ["out"], (in_names, out_names)
        all_in_names = in_names + out_names
        if partition_name is not None:
            all_in_names = all_in_names + [partition_name]

        def _body(*args):
            operands = list(args)
            if partition_name is not None:
                operands.append(bass2jax.partition_id_tensor())
            return tuple(bass2jax._bass_exec_p.bind(
                *operands,
                out_avals=tuple(out_avals),
                in_names=tuple(all_in_names),
                out_names=tuple(out_names),
                lowering_input_output_aliases=(),
                sim_require_finite=True,
                sim_require_nnan=True,
                nc=nc,
            ))

        devices = jax.devices()[:N_CORES]
        assert len(devices) == N_CORES
        self.mesh = Mesh(np.asarray(devices), ("core",))
        spec = PartitionSpec("core")
        self.sharding = NamedSharding(self.mesh, spec)
        self.jitted = jax.jit(shard_map(
            _body, mesh=self.mesh, in_specs=(spec, spec), out_specs=(spec,),
            check_rep=False))
        # device-resident dummy output operand, created once
        self.zeros_dev = jax.device_put(
            np.zeros((N_CORES * LB, Z, A, K, NCLS), np.float32), self.sharding)
        self._jax = jax

    def put(self, x: np.ndarray):
        return self._jax.device_put(
            np.ascontiguousarray(np.asarray(x, np.float32)), self.sharding)

    def run_dev(self, x_dev):
        """Execute; returns device array (not fetched)."""
        return self.jitted(x_dev, self.zeros_dev)[0]

    def __call__(self, x: np.ndarray) -> np.ndarray:
        return np.asarray(self.run_dev(self.put(x)))


_RUNNER = None


def _get_runner():
    global _RUNNER
    if _RUNNER is None:
        _RUNNER = _Runner()
    return _RUNNER


def kernel(x: np.ndarray) -> np.ndarray:
    x = np.asarray(x, dtype=np.float32)
    assert x.shape == (B, Z, A, C), x.shape
    try:
        return _get_runner()(x)
    except Exception:
        # fallback: reference-quality but slower dispatch path
        nc = _build()
        in_maps = [{"x": np.ascontiguousarray(x[i * LB:(i + 1) * LB])}
                   for i in range(N_CORES)]
        res = run_bass_kernel_spmd(nc, in_maps, list(range(N_CORES)))
        return np.concatenate(
            [res.results[i]["out"] for i in range(N_CORES)], axis=0)


# revision 21
# speedup vs baseline: 1826.6445x; 1773.1160x over previous
"""BilateralFilter (SqueezeSeg mc condensing-kernel gaussians) on 8 TRN2 cores.

Reference computes, for x: [16, 64, 512, 3] (B, Z, A, C=xyz):
    nbr   = 14 spatial neighbors of each pixel in a 3x5 window (zero-padded)
    diff2 = sum_c (x - nbr)^2                           [B, Z, A, 14]
    out   = exp(-diff2 / (2 * theta_r^2))               [B, Z, A, 14, 4]
with THETA_R = [0.015, 0.015, 0.01, 0.01] (only 2 distinct values).

Strategy (pure batch data-parallel, 2 batches per core):
  - partitions p = b*64 + z  (128), free dim = azimuth chunks of 256 (x2).
  - mirror symmetry: m_k(q) = |x(q) - x(q+off_k)|^2 for the 7 "negative"
    offsets k=0..6 gives the other 7 via diff2_{13-k}(q) = m_k(q - off_k);
    the z+1-partition read is materialized by an SBUF->SBUF partition-remap
    DMA (M_up), with the phantom z=64 row filled from s = |x|^2 by a
    stride-0 DMA (out-of-image neighbor => diff2 = |x(center)|^2).
  - ACT computes exp with the free scale immediate; each exp is written to
    both classes of its theta pair via a stride-0 input axis.
  - output staged in SBUF exactly in DRAM layout [a, k, c] so the store DMA
    is fully contiguous per partition (57 KB/partition runs).
"""

import numpy as np

import concourse.bass as bass
import concourse.tile as tile
from concourse import bacc, mybir
from concourse.bass_utils import run_bass_kernel_spmd

N_CORES = 8
B, Z, A, C = 16, 64, 512, 3
K, NCLS = 14, 4
LB = B // N_CORES            # local batches per core = 2
P = LB * Z                   # 128 partitions
AC = 256                     # azimuth chunk
BUFS = 2                     # tile pool buffers
F32 = mybir.dt.float32

# exp scales: -1 / (2 * theta^2), theta pairs (0.015, 0.01), f32 semantics
_t0 = np.float32(0.015)
_t1 = np.float32(0.01)
SC0 = -float(1.0 / np.float32(np.float32(2.0) * _t0 * _t0))
SC1 = -float(1.0 / np.float32(np.float32(2.0) * _t1 * _t1))

# DRAM strides (elements) of out [LB, Z, A, K, NCLS]
O_A = K * NCLS               # 56
O_Z = A * O_A                # 28672
O_B = Z * O_Z                # 1835008
X_Z = A * C                  # 1536
X_B = Z * X_Z


def _ap(t, poff, pcnt, foff, pairs, pstep=1):
    """AP on tile t: partitions [poff, poff+pcnt) (stride pstep rows), free
    `pairs` ([step, count] in elements) based at element foff."""
    row = t.ap[0][0]
    return bass.AP(tensor=t.tensor, offset=t.offset + poff * row + foff,
                   ap=[[pstep * row, pcnt]] + [list(p) for p in pairs])


_SQDIFF = None


def _get_sqdiff():
    """Register a runtime custom DVE op: out = (in0 - in1)^2 (fp32, one
    instruction instead of subtract + multiply)."""
    global _SQDIFF
    if _SQDIFF is not None:
        return _SQDIFF
    from concourse import dve_ops
    from concourse.dve_spec import Spec, Src0, Src1, sq, lower, _has_src1
    from concourse.dve_uop import DveOpSpec

    name = "SQDIFF_BILAT_ANT"
    if name not in dve_ops._SUB_OPCODE_FOR_NAME:
        spec = Spec(
            body=sq(Src0 - Src1),
            reference=lambda in0, in1, c0, c1, c2:
                (in0.astype(np.float32) - in1.astype(np.float32)) ** 2)
        row = 1 + len(dve_ops.OPS)
        assert row < 0x20
        shas = {}
        for ver in ("v3",):
            tmp = DveOpSpec(name=name, opcode=row, uops=lower(spec, ver=ver),
                            rd1_en=_has_src1(spec))
            shas[ver] = tmp.sha(ver)
        op = dve_ops.DveOp(name, spec, subdim=False, uops_sha=shas)
        dve_ops.OPS.append(op)
        dve_ops.CUSTOM_DVE_SPECS[name] = spec
        dve_ops._SUB_OPCODE_FOR_NAME[name] = row
    else:
        op = next(o for o in dve_ops.OPS if o.name == name)
    _SQDIFF = op
    return op


def _build(ac=AC, bufs=BUFS, reps=1):
    XW = ac + 8                  # x window (halo 4 each side)
    MW = ac + 4                  # m window (halo 2 each side)
    NCH = A // ac
    nc = bacc.Bacc("TRN2", target_bir_lowering=False, debug=False,
                   num_devices=N_CORES)
    x_h = nc.dram_tensor("x", [LB, Z, A, C], F32, kind="ExternalInput")
    o_h = nc.dram_tensor("out", [LB, Z, A, K, NCLS], F32, kind="ExternalOutput")
    x_ap, o_ap = x_h.ap(), o_h.ap()
    # bench mode: reps > 1 re-runs the whole kernel; each non-final pass
    # stores to its own DRAM scratch so stores are real traffic
    scratch_aps = [
        nc.dram_tensor(f"scr{r}", [LB, Z, A, K, NCLS], F32).ap()
        for r in range(reps - 1)]

    with tile.TileContext(nc) as tc:
        with tc.tile_pool(name="pool", bufs=bufs) as pool:
            for ci in range(NCH * reps):
                a0 = (ci % NCH) * ac
                lo, hi = max(0, a0 - 4), min(A, a0 + ac + 4)
                c_lo = (lo - (a0 - 4)) * C          # first valid xt col
                c_hi = (hi - (a0 - 4)) * C

                # ---- load x window (zero halo at image borders) ----
                # (b, z) rows are contiguous in DRAM: one 128-partition DMA
                xt = pool.tile([P, XW * C], F32, name="xt")
                if c_lo > 0:
                    nc.gpsimd.memset(_ap(xt, 0, P, 0, [[1, c_lo]]), 0.0)
                if c_hi < XW * C:
                    nc.gpsimd.memset(
                        _ap(xt, 0, P, c_hi, [[1, XW * C - c_hi]]), 0.0)
                nc.sync.dma_start(
                    _ap(xt, 0, P, c_lo, [[C, hi - lo], [1, C]]),
                    bass.AP(tensor=x_ap.tensor, offset=lo * C,
                            ap=[[X_Z, P], [C, hi - lo], [1, C]]))

                # ---- x_dn[p] = x at (z-1) (zeros at z=0 rows) ----
                # one DMA for rows 1..127 (row 64 gets cross-batch garbage,
                # re-zeroed after), plus halo/row-0 zeros from the memset
                x_dn = pool.tile([P, XW * C], F32, name="x_dn")
                nc.gpsimd.memset(x_dn[:], 0.0)
                for b in range(LB):
                    nc.sync.dma_start(
                        _ap(x_dn, b * Z + 1, Z - 1, c_lo, [[C, hi - lo], [1, C]]),
                        bass.AP(tensor=x_ap.tensor, offset=b * X_B + lo * C,
                                ap=[[X_Z, Z - 1], [C, hi - lo], [1, C]]))

                # ---- s = sum_c x^2 over the full x window ----
                sqx = pool.tile([P, XW * C], F32, name="sqx")
                nc.scalar.square(sqx[:], xt[:])
                st = pool.tile([P, XW], F32, name="st")
                nc.vector.tensor_reduce(
                    st[:], _ap(sqx, 0, P, 0, [[C, XW], [1, C]]),
                    axis=mybir.AxisListType.X, op=mybir.AluOpType.add)

                # ---- m_k maps: M[p, k*MW + ar] over a-window [a0-2, a0+258)
                # k=0..4: dz=-1, da=k-2 ; k=5,6: dz=0, da=k-7
                # d2 = (x - x_nbr)^2 in one fused custom op per k
                sqdiff = _get_sqdiff()
                M = pool.tile([P, 7 * MW], F32, name="M")
                dt5 = pool.tile([P, 5 * MW * C], F32, name="dt5")
                for k in range(5):
                    nc.vector._custom_dve(
                        sqdiff,
                        out=_ap(dt5, 0, P, k * MW * C, [[C, MW], [1, C]]),
                        in0=_ap(xt, 0, P, 2 * C, [[C, MW], [1, C]]),
                        in1=_ap(x_dn, 0, P, k * C, [[C, MW], [1, C]]))
                nc.vector.tensor_reduce(
                    _ap(M, 0, P, 0, [[1, 5 * MW]]),
                    _ap(dt5, 0, P, 0, [[C, 5 * MW], [1, C]]),
                    axis=mybir.AxisListType.X, op=mybir.AluOpType.add)

                dt2 = pool.tile([P, 2 * MW * C], F32, name="dt2")
                for k in (5, 6):
                    nc.vector._custom_dve(
                        sqdiff,
                        out=_ap(dt2, 0, P, (k - 5) * MW * C, [[C, MW], [1, C]]),
                        in0=_ap(xt, 0, P, 2 * C, [[C, MW], [1, C]]),
                        in1=_ap(xt, 0, P, (k - 5) * C, [[C, MW], [1, C]]))
                nc.vector.tensor_reduce(
                    _ap(M, 0, P, 5 * MW, [[1, 2 * MW]]),
                    _ap(dt2, 0, P, 0, [[C, 2 * MW], [1, C]]),
                    axis=mybir.AxisListType.X, op=mybir.AluOpType.add)

                # ---- M_up[p] = M[p+1] for k=0..4 cols; phantom z=64 rows
                # ({63,127}) = s(z=63 row) with k-dependent a-shift ----
                M_up = pool.tile([P, 5 * MW], F32, name="M_up")
                # disjoint remaps per batch so the phantom DMA runs parallel
                for b in range(LB):
                    nc.sync.dma_start(
                        _ap(M_up, b * Z, Z - 1, 0, [[1, 5 * MW]]),
                        _ap(M, b * Z + 1, Z - 1, 0, [[1, 5 * MW]]))
                # phantom: M_up[{63,127}, k*MW + ar] = st[{63,127}, ar + k]
                nc.sync.dma_start(
                    _ap(M_up, Z - 1, 2, 0, [[MW, 5], [1, MW]], pstep=Z),
                    _ap(st, Z - 1, 2, 0, [[1, 5], [1, MW]], pstep=Z))

                # ---- exps into O staging [p, ar*56 + k*4 + c] ----
                O = pool.tile([P, ac * O_A], F32, name="O",
                              bufs=(1 if ac >= 512 else None))
                for th, sc in ((0, SC0), (1, SC1)):
                    co = 2 * th
                    # direct k=0..6: in M[p, k*MW + ar + 2]
                    nc.scalar.activation(
                        _ap(O, 0, P, co, [[4, 7], [O_A, ac], [1, 2]]),
                        _ap(M, 0, P, 2, [[MW, 7], [1, ac], [0, 2]]),
                        mybir.ActivationFunctionType.Exp, scale=sc)
                    # a-mirrors k'=7,8 <- k=6,5: col = k*MW + ar + (9-k)
                    nc.scalar.activation(
                        _ap(O, 0, P, 28 + co, [[4, 2], [O_A, ac], [1, 2]]),
                        _ap(M, 0, P, 6 * MW + 3, [[-(MW - 1), 2], [1, ac], [0, 2]]),
                        mybir.ActivationFunctionType.Exp, scale=sc)
                    # dz-mirrors k'=9..13 <- k=4..0: M_up[p, k*MW + ar + 4 - k]
                    nc.scalar.activation(
                        _ap(O, 0, P, 36 + co, [[4, 5], [O_A, ac], [1, 2]]),
                        _ap(M_up, 0, P, 4 * (MW - 1) + 4,
                            [[-(MW - 1), 5], [1, ac], [0, 2]]),
                        mybir.ActivationFunctionType.Exp, scale=sc)

                # ---- store: one contiguous 128-partition DMA ----
                rep_i = ci // NCH
                dst_ap = o_ap if rep_i == reps - 1 else scratch_aps[rep_i]
                nc.sync.dma_start(
                    bass.AP(tensor=dst_ap.tensor, offset=a0 * O_A,
                            ap=[[O_Z, P], [1, ac * O_A]]),
                    _ap(O, 0, P, 0, [[1, ac * O_A]]))

    nc.compile()
    return nc


class _Runner:
    """Compile once; reuse the jitted sharded executable across calls.

    Mirrors bass2jax.run_bass_via_pjrt's multi-core path, but without
    donated output buffers (the kernel writes every output element, so the
    zero "output operands" are passed once from device-resident buffers and
    reused)."""

    def __init__(self):
        import jax
        from jax.sharding import Mesh, PartitionSpec, NamedSharding
        try:
            from jax.experimental.shard_map import shard_map
        except ImportError:
            from jax.shard_map import shard_map  # newer jax
        from concourse import bass2jax

        bass2jax.install_neuronx_cc_hook()
        nc = _build()
        self.nc = nc

        partition_name = (nc.partition_id_tensor.name
                          if nc.partition_id_tensor else None)
        in_names, out_names, out_avals = [], [], []
        for alloc in nc.m.functions[0].allocations:
            if not isinstance(alloc, mybir.MemoryLocationSet):
                continue
            name = alloc.memorylocations[0].name
            if alloc.kind == "ExternalInput":
                if name != partition_name:
                    in_names.append(name)
            elif alloc.kind == "ExternalOutput":
                out_names.append(name)
                out_avals.append(jax.core.ShapedArray(
                    tuple(alloc.tensor_shape), mybir.dt.np(alloc.dtype)))
        assert in_names == ["x"] and out_names == ["out"], (in_names, out_names)
        all_in_names = in_names + out_names
        if partition_name is not None:
            all_in_names = all_in_names + [partition_name]

        def _body(*args):
            operands = list(args)
            if partition_name is not None:
                operands.append(bass2jax.partition_id_tensor())
            return tuple(bass2jax._bass_exec_p.bind(
                *operands,
                out_avals=tuple(out_avals),
                in_names=tuple(all_in_names),
                out_names=tuple(out_names),
                lowering_input_output_aliases=(),
                sim_require_finite=True,
                sim_require_nnan=True,
                nc=nc,
            ))

        devices = jax.devices()[:N_CORES]
        assert len(devices) == N_CORES
        self.mesh = Mesh(np.asarray(devices), ("core",))
        spec = PartitionSpec("core")
        self.sharding = NamedSharding(self.mesh, spec)
        self.jitted = jax.jit(shard_map(
            _body, mesh=self.mesh, in_specs=(spec, spec), out_specs=(spec,),
            check_rep=False))
        # device-resident dummy output operand, created once
        self.zeros_dev = jax.device_put(
            np.zeros((N_CORES * LB, Z, A, K, NCLS), np.float32), self.sharding)
        self._jax = jax

    def put(self, x: np.ndarray):
        return self._jax.device_put(
            np.ascontiguousarray(np.asarray(x, np.float32)), self.sharding)

    def run_dev(self, x_dev):
        """Execute; returns device array (not fetched)."""
        return self.jitted(x_dev, self.zeros_dev)[0]

    def __call__(self, x: np.ndarray) -> np.ndarray:
        return np.asarray(self.run_dev(self.put(x)))


_RUNNER = None


def _get_runner():
    global _RUNNER
    if _RUNNER is None:
        _RUNNER = _Runner()
    return _RUNNER


def kernel(x: np.ndarray) -> np.ndarray:
    x = np.asarray(x, dtype=np.float32)
    assert x.shape == (B, Z, A, C), x.shape
    try:
        return _get_runner()(x)
    except Exception:
        # fallback: reference-quality but slower dispatch path
        nc = _build()
        in_maps = [{"x": np.ascontiguousarray(x[i * LB:(i + 1) * LB])}
                   for i in range(N_CORES)]
        res = run_bass_kernel_spmd(nc, in_maps, list(range(N_CORES)))
        return np.concatenate(
            [res.results[i]["out"] for i in range(N_CORES)], axis=0)


# revision 23
# speedup vs baseline: 1953.2162x; 1.0693x over previous
"""BilateralFilter (SqueezeSeg mc condensing-kernel gaussians) on 8 TRN2 cores.

Reference computes, for x: [16, 64, 512, 3] (B, Z, A, C=xyz):
    nbr   = 14 spatial neighbors of each pixel in a 3x5 window (zero-padded)
    diff2 = sum_c (x - nbr)^2                           [B, Z, A, 14]
    out   = exp(-diff2 / (2 * theta_r^2))               [B, Z, A, 14, 4]
with THETA_R = [0.015, 0.015, 0.01, 0.01] (only 2 distinct values).

Strategy (pure batch data-parallel, 2 batches per core):
  - partitions p = b*64 + z  (128), free dim = azimuth chunks (AC wide).
  - mirror symmetry: m_k(q) = |x(q) - x(q+off_k)|^2 for the 7 "negative"
    offsets k=0..6 gives the other 7 via diff2_{13-k}(q) = m_k(q - off_k);
    the z+1-partition read is materialized by an SBUF->SBUF partition-remap
    DMA (M_up), with the phantom z=64 row filled from s = |x|^2 by a
    stride-0 DMA (out-of-image neighbor => diff2 = |x(center)|^2).
  - ACT computes exp with the free scale immediate; each exp is written to
    both classes of its theta pair via a stride-0 input axis.
  - output staged in SBUF exactly in DRAM layout [a, k, c] so the store DMA
    is fully contiguous per partition (57 KB/partition runs).
"""

import numpy as np

import concourse.bass as bass
import concourse.tile as tile
from concourse import bacc, mybir
from concourse.bass_utils import run_bass_kernel_spmd

N_CORES = 8
B, Z, A, C = 16, 64, 512, 3
K, NCLS = 14, 4
LB = B // N_CORES            # local batches per core = 2
P = LB * Z                   # 128 partitions
AC = 128                     # azimuth chunk
BUFS = 3                     # tile pool buffers
F32 = mybir.dt.float32

# exp scales: -1 / (2 * theta^2), theta pairs (0.015, 0.01), f32 semantics
_t0 = np.float32(0.015)
_t1 = np.float32(0.01)
SC0 = -float(1.0 / np.float32(np.float32(2.0) * _t0 * _t0))
SC1 = -float(1.0 / np.float32(np.float32(2.0) * _t1 * _t1))

# DRAM strides (elements) of out [LB, Z, A, K, NCLS]
O_A = K * NCLS               # 56
O_Z = A * O_A                # 28672
O_B = Z * O_Z                # 1835008
X_Z = A * C                  # 1536
X_B = Z * X_Z


def _ap(t, poff, pcnt, foff, pairs, pstep=1):
    """AP on tile t: partitions [poff, poff+pcnt) (stride pstep rows), free
    `pairs` ([step, count] in elements) based at element foff."""
    row = t.ap[0][0]
    return bass.AP(tensor=t.tensor, offset=t.offset + poff * row + foff,
                   ap=[[pstep * row, pcnt]] + [list(p) for p in pairs])


_SQDIFF = None


def _get_sqdiff():
    """Register a runtime custom DVE op: out = (in0 - in1)^2 (fp32, one
    instruction instead of subtract + multiply)."""
    global _SQDIFF
    if _SQDIFF is not None:
        return _SQDIFF
    from concourse import dve_ops
    from concourse.dve_spec import Spec, Src0, Src1, sq, lower, _has_src1
    from concourse.dve_uop import DveOpSpec

    name = "SQDIFF_BILAT_ANT"
    if name not in dve_ops._SUB_OPCODE_FOR_NAME:
        spec = Spec(
            body=sq(Src0 - Src1),
            reference=lambda in0, in1, c0, c1, c2:
                (in0.astype(np.float32) - in1.astype(np.float32)) ** 2)
        row = 1 + len(dve_ops.OPS)
        assert row < 0x20
        shas = {}
        for ver in ("v3",):
            tmp = DveOpSpec(name=name, opcode=row, uops=lower(spec, ver=ver),
                            rd1_en=_has_src1(spec))
            shas[ver] = tmp.sha(ver)
        op = dve_ops.DveOp(name, spec, subdim=False, uops_sha=shas)
        dve_ops.OPS.append(op)
        dve_ops.CUSTOM_DVE_SPECS[name] = spec
        dve_ops._SUB_OPCODE_FOR_NAME[name] = row
    else:
        op = next(o for o in dve_ops.OPS if o.name == name)
    _SQDIFF = op
    return op


def _build(ac=AC, bufs=BUFS, reps=1):
    XW = ac + 8                  # x window (halo 4 each side)
    MW = ac + 4                  # m window (halo 2 each side)
    NCH = A // ac
    nc = bacc.Bacc("TRN2", target_bir_lowering=False, debug=False,
                   num_devices=N_CORES)
    x_h = nc.dram_tensor("x", [LB, Z, A, C], F32, kind="ExternalInput")
    o_h = nc.dram_tensor("out", [LB, Z, A, K, NCLS], F32, kind="ExternalOutput")
    x_ap, o_ap = x_h.ap(), o_h.ap()
    # bench mode: reps > 1 re-runs the whole kernel; each non-final pass
    # stores to its own DRAM scratch so stores are real traffic
    scratch_aps = [
        nc.dram_tensor(f"scr{r}", [LB, Z, A, K, NCLS], F32).ap()
        for r in range(reps - 1)]

    with tile.TileContext(nc) as tc:
        with tc.tile_pool(name="pool", bufs=bufs) as pool:
            for ci in range(NCH * reps):
                a0 = (ci % NCH) * ac
                lo, hi = max(0, a0 - 4), min(A, a0 + ac + 4)
                c_lo = (lo - (a0 - 4)) * C          # first valid xt col
                c_hi = (hi - (a0 - 4)) * C

                # ---- load x window (zero halo at image borders) ----
                # (b, z) rows are contiguous in DRAM: one 128-partition DMA
                xt = pool.tile([P, XW * C], F32, name="xt")
                if c_lo > 0:
                    nc.gpsimd.memset(_ap(xt, 0, P, 0, [[1, c_lo]]), 0.0)
                if c_hi < XW * C:
                    nc.gpsimd.memset(
                        _ap(xt, 0, P, c_hi, [[1, XW * C - c_hi]]), 0.0)
                nc.sync.dma_start(
                    _ap(xt, 0, P, c_lo, [[C, hi - lo], [1, C]]),
                    bass.AP(tensor=x_ap.tensor, offset=lo * C,
                            ap=[[X_Z, P], [C, hi - lo], [1, C]]))

                # ---- x_dn[p] = x at (z-1) (zeros at z=0 rows) ----
                # one DMA for rows 1..127 (row 64 gets cross-batch garbage,
                # re-zeroed after), plus halo/row-0 zeros from the memset
                x_dn = pool.tile([P, XW * C], F32, name="x_dn")
                nc.gpsimd.memset(x_dn[:], 0.0)
                for b in range(LB):
                    nc.sync.dma_start(
                        _ap(x_dn, b * Z + 1, Z - 1, c_lo, [[C, hi - lo], [1, C]]),
                        bass.AP(tensor=x_ap.tensor, offset=b * X_B + lo * C,
                                ap=[[X_Z, Z - 1], [C, hi - lo], [1, C]]))

                # ---- s = sum_c x^2 over the full x window ----
                sqx = pool.tile([P, XW * C], F32, name="sqx")
                nc.scalar.square(sqx[:], xt[:])
                st = pool.tile([P, XW], F32, name="st")
                nc.vector.tensor_reduce(
                    st[:], _ap(sqx, 0, P, 0, [[C, XW], [1, C]]),
                    axis=mybir.AxisListType.X, op=mybir.AluOpType.add)

                # ---- m_k maps: M[p, k*MW + ar] over a-window [a0-2, a0+258)
                # k=0..4: dz=-1, da=k-2 ; k=5,6: dz=0, da=k-7
                # d2 = (x - x_nbr)^2 in one fused custom op per k
                sqdiff = _get_sqdiff()
                M = pool.tile([P, 7 * MW], F32, name="M")
                dt5 = pool.tile([P, 5 * MW * C], F32, name="dt5")
                for k in range(5):
                    nc.vector._custom_dve(
                        sqdiff,
                        out=_ap(dt5, 0, P, k * MW * C, [[C, MW], [1, C]]),
                        in0=_ap(xt, 0, P, 2 * C, [[C, MW], [1, C]]),
                        in1=_ap(x_dn, 0, P, k * C, [[C, MW], [1, C]]))
                nc.vector.tensor_reduce(
                    _ap(M, 0, P, 0, [[1, 5 * MW]]),
                    _ap(dt5, 0, P, 0, [[C, 5 * MW], [1, C]]),
                    axis=mybir.AxisListType.X, op=mybir.AluOpType.add)

                dt2 = pool.tile([P, 2 * MW * C], F32, name="dt2")
                for k in (5, 6):
                    nc.vector._custom_dve(
                        sqdiff,
                        out=_ap(dt2, 0, P, (k - 5) * MW * C, [[C, MW], [1, C]]),
                        in0=_ap(xt, 0, P, 2 * C, [[C, MW], [1, C]]),
                        in1=_ap(xt, 0, P, (k - 5) * C, [[C, MW], [1, C]]))
                nc.vector.tensor_reduce(
                    _ap(M, 0, P, 5 * MW, [[1, 2 * MW]]),
                    _ap(dt2, 0, P, 0, [[C, 2 * MW], [1, C]]),
                    axis=mybir.AxisListType.X, op=mybir.AluOpType.add)

                # ---- M_up[p] = M[p+1] for k=0..4 cols; phantom z=64 rows
                # ({63,127}) = s(z=63 row) with k-dependent a-shift ----
                M_up = pool.tile([P, 5 * MW], F32, name="M_up")
                # disjoint remaps per batch so the phantom DMA runs parallel
                for b in range(LB):
                    nc.sync.dma_start(
                        _ap(M_up, b * Z, Z - 1, 0, [[1, 5 * MW]]),
                        _ap(M, b * Z + 1, Z - 1, 0, [[1, 5 * MW]]))
                # phantom: M_up[{63,127}, k*MW + ar] = st[{63,127}, ar + k]
                nc.sync.dma_start(
                    _ap(M_up, Z - 1, 2, 0, [[MW, 5], [1, MW]], pstep=Z),
                    _ap(st, Z - 1, 2, 0, [[1, 5], [1, MW]], pstep=Z))

                # ---- exps into O staging [p, ar*56 + k*4 + c] ----
                O = pool.tile([P, ac * O_A], F32, name="O",
                              bufs=(1 if ac >= 512 else None))
                for th, sc in ((0, SC0), (1, SC1)):
                    co = 2 * th
                    # direct k=0..6: in M[p, k*MW + ar + 2]
                    nc.scalar.activation(
                        _ap(O, 0, P, co, [[4, 7], [O_A, ac], [1, 2]]),
                        _ap(M, 0, P, 2, [[MW, 7], [1, ac], [0, 2]]),
                        mybir.ActivationFunctionType.Exp, scale=sc)
                    # a-mirrors k'=7,8 <- k=6,5: col = k*MW + ar + (9-k)
                    nc.scalar.activation(
                        _ap(O, 0, P, 28 + co, [[4, 2], [O_A, ac], [1, 2]]),
                        _ap(M, 0, P, 6 * MW + 3, [[-(MW - 1), 2], [1, ac], [0, 2]]),
                        mybir.ActivationFunctionType.Exp, scale=sc)
                    # dz-mirrors k'=9..13 <- k=4..0: M_up[p, k*MW + ar + 4 - k]
                    nc.scalar.activation(
                        _ap(O, 0, P, 36 + co, [[4, 5], [O_A, ac], [1, 2]]),
                        _ap(M_up, 0, P, 4 * (MW - 1) + 4,
                            [[-(MW - 1), 5], [1, ac], [0, 2]]),
                        mybir.ActivationFunctionType.Exp, scale=sc)

                # ---- store: one contiguous 128-partition DMA ----
                rep_i = ci // NCH
                dst_ap = o_ap if rep_i == reps - 1 else scratch_aps[rep_i]
                nc.sync.dma_start(
                    bass.AP(tensor=dst_ap.tensor, offset=a0 * O_A,
                            ap=[[O_Z, P], [1, ac * O_A]]),
                    _ap(O, 0, P, 0, [[1, ac * O_A]]))

    nc.compile()
    return nc


class _Runner:
    """Compile once; reuse the jitted sharded executable across calls.

    Mirrors bass2jax.run_bass_via_pjrt's multi-core path, but without
    donated output buffers (the kernel writes every output element, so the
    zero "output operands" are passed once from device-resident buffers and
    reused)."""

    def __init__(self):
        import jax
        from jax.sharding import Mesh, PartitionSpec, NamedSharding
        try:
            from jax.experimental.shard_map import shard_map
        except ImportError:
            from jax.shard_map import shard_map  # newer jax
        from concourse import bass2jax

        bass2jax.install_neuronx_cc_hook()
        nc = _build()
        self.nc = nc

        partition_name = (nc.partition_id_tensor.name
                          if nc.partition_id_tensor else None)
        in_names, out_names, out_avals = [], [], []
        for alloc in nc.m.functions[0].allocations:
            if not isinstance(alloc, mybir.MemoryLocationSet):
                continue
            name = alloc.memorylocations[0].name
            if alloc.kind == "ExternalInput":
                if name != partition_name:
                    in_names.append(name)
            elif alloc.kind == "ExternalOutput":
                out_names.append(name)
                out_avals.append(jax.core.ShapedArray(
                    tuple(alloc.tensor_shape), mybir.dt.np(alloc.dtype)))
        assert in_names == ["x"] and out_names == ["out"], (in_names, out_names)
        all_in_names = in_names + out_names
        if partition_name is not None:
            all_in_names = all_in_names + [partition_name]

        def _body(*args):
            operands = list(args)
            if partition_name is not None:
                operands.append(bass2jax.partition_id_tensor())
            return tuple(bass2jax._bass_exec_p.bind(
                *operands,
                out_avals=tuple(out_avals),
                in_names=tuple(all_in_names),
                out_names=tuple(out_names),
                lowering_input_output_aliases=(),
                sim_require_finite=True,
                sim_require_nnan=True,
                nc=nc,
            ))

        devices = jax.devices()[:N_CORES]
        assert len(devices) == N_CORES
        self.mesh = Mesh(np.asarray(devices), ("core",))
        spec = PartitionSpec("core")
        self.sharding = NamedSharding(self.mesh, spec)
        self.jitted = jax.jit(shard_map(
            _body, mesh=self.mesh, in_specs=(spec, spec), out_specs=(spec,),
            check_rep=False))
        # device-resident dummy output operand, created once
        self.zeros_dev = jax.device_put(
            np.zeros((N_CORES * LB, Z, A, K, NCLS), np.float32), self.sharding)
        self._jax = jax

    def put(self, x: np.ndarray):
        return self._jax.device_put(
            np.ascontiguousarray(np.asarray(x, np.float32)), self.sharding)

    def run_dev(self, x_dev):
        """Execute; returns device array (not fetched)."""
        return self.jitted(x_dev, self.zeros_dev)[0]

    def __call__(self, x: np.ndarray) -> np.ndarray:
        return np.asarray(self.run_dev(self.put(x)))


_RUNNER = None


def _get_runner():
    global _RUNNER
    if _RUNNER is None:
        _RUNNER = _Runner()
    return _RUNNER


def kernel(x: np.ndarray) -> np.ndarray:
    x = np.asarray(x, dtype=np.float32)
    assert x.shape == (B, Z, A, C), x.shape
    try:
        return _get_runner()(x)
    except Exception:
        # fallback: reference-quality but slower dispatch path
        nc = _build()
        in_maps = [{"x": np.ascontiguousarray(x[i * LB:(i + 1) * LB])}
                   for i in range(N_CORES)]
        res = run_bass_kernel_spmd(nc, in_maps, list(range(N_CORES)))
        return np.concatenate(
            [res.results[i]["out"] for i in range(N_CORES)], axis=0)


# revision 34
# speedup vs baseline: 3498.8278x; 1.7913x over previous
"""BilateralFilter (SqueezeSeg mc condensing-kernel gaussians) on 8 TRN2 cores.

Reference computes, for x: [16, 64, 512, 3] (B, Z, A, C=xyz):
    nbr   = 14 spatial neighbors of each pixel in a 3x5 window (zero-padded)
    diff2 = sum_c (x - nbr)^2                           [B, Z, A, 14]
    out   = exp(-diff2 / (2 * theta_r^2))               [B, Z, A, 14, 4]
with THETA_R = [0.015, 0.015, 0.01, 0.01] (only 2 distinct values).

Strategy (pure batch data-parallel, 2 batches per core):
  - partitions p = b*64 + z  (128), free dim = azimuth chunks (AC wide).
  - mirror symmetry: m_k(q) = |x(q) - x(q+off_k)|^2 for the 7 "negative"
    offsets k=0..6 gives the other 7 via diff2_{13-k}(q) = m_k(q - off_k);
    the z+1-partition read is materialized by an SBUF->SBUF partition-remap
    DMA (M_up), with the phantom z=64 row filled from s = |x|^2 by a
    stride-0 DMA (out-of-image neighbor => diff2 = |x(center)|^2).
  - ACT computes exp with the free scale immediate; each exp is written to
    both classes of its theta pair via a stride-0 input axis.
  - output staged in SBUF exactly in DRAM layout [a, k, c] so the store DMA
    is fully contiguous per partition (57 KB/partition runs).
"""

import numpy as np

import concourse.bass as bass
import concourse.tile as tile
from concourse import bacc, mybir
from concourse.bass_utils import run_bass_kernel_spmd

N_CORES = 8
B, Z, A, C = 16, 64, 512, 3
K, NCLS = 14, 4
LB = B // N_CORES            # local batches per core = 2
P = LB * Z                   # 128 partitions
AC = 128                     # azimuth chunk
BUFS = 3                     # tile pool buffers
PE_SHIFT = True              # z+1 partition shift via PE matmul vs SBUF DMA
F32 = mybir.dt.float32


def _host_shift_mats():
    """SH2[k, m] = 1 iff k == m+1 (and not m == 63: batch boundary);
    SEL[k, m] = 1 iff k == m in {63, 127} (phantom z=64 row selector)."""
    sh = np.zeros((P, P), np.float32)
    for m in range(P - 1):
        if m != Z - 1:
            sh[m + 1, m] = 1.0
    sel = np.zeros((P, P), np.float32)
    sel[Z - 1, Z - 1] = 1.0
    sel[P - 1, P - 1] = 1.0
    return sh, sel

# exp scales: -1 / (2 * theta^2), theta pairs (0.015, 0.01), f32 semantics
_t0 = np.float32(0.015)
_t1 = np.float32(0.01)
SC0 = -float(1.0 / np.float32(np.float32(2.0) * _t0 * _t0))
SC1 = -float(1.0 / np.float32(np.float32(2.0) * _t1 * _t1))

# DRAM strides (elements) of out [LB, Z, A, K, NCLS]
O_A = K * NCLS               # 56
O_Z = A * O_A                # 28672
O_B = Z * O_Z                # 1835008
X_Z = A * C                  # 1536
X_B = Z * X_Z


def _ap(t, poff, pcnt, foff, pairs, pstep=1):
    """AP on tile t: partitions [poff, poff+pcnt) (stride pstep rows), free
    `pairs` ([step, count] in elements) based at element foff."""
    row = t.ap[0][0]
    return bass.AP(tensor=t.tensor, offset=t.offset + poff * row + foff,
                   ap=[[pstep * row, pcnt]] + [list(p) for p in pairs])


_SQDIFF = None


def _get_sqdiff():
    """Register a runtime custom DVE op: out = (in0 - in1)^2 (fp32, one
    instruction instead of subtract + multiply)."""
    global _SQDIFF
    if _SQDIFF is not None:
        return _SQDIFF
    from concourse import dve_ops
    from concourse.dve_spec import Spec, Src0, Src1, sq, lower, _has_src1
    from concourse.dve_uop import DveOpSpec

    name = "SQDIFF_BILAT_ANT"
    if name not in dve_ops._SUB_OPCODE_FOR_NAME:
        spec = Spec(
            body=sq(Src0 - Src1),
            reference=lambda in0, in1, c0, c1, c2:
                (in0.astype(np.float32) - in1.astype(np.float32)) ** 2)
        row = 1 + len(dve_ops.OPS)
        assert row < 0x20
        shas = {}
        for ver in ("v3",):
            tmp = DveOpSpec(name=name, opcode=row, uops=lower(spec, ver=ver),
                            rd1_en=_has_src1(spec))
            shas[ver] = tmp.sha(ver)
        op = dve_ops.DveOp(name, spec, subdim=False, uops_sha=shas)
        dve_ops.OPS.append(op)
        dve_ops.CUSTOM_DVE_SPECS[name] = spec
        dve_ops._SUB_OPCODE_FOR_NAME[name] = row
    else:
        op = next(o for o in dve_ops.OPS if o.name == name)
    _SQDIFF = op
    return op


def _build(ac=AC, bufs=BUFS, reps=1, pe_shift=PE_SHIFT):
    XW = ac + 8                  # x window (halo 4 each side)
    MW = ac + 4                  # m window (halo 2 each side)
    NCH = A // ac
    nc = bacc.Bacc("TRN2", target_bir_lowering=False, debug=False,
                   num_devices=N_CORES)
    x_h = nc.dram_tensor("x", [LB, Z, A, C], F32, kind="ExternalInput")
    o_h = nc.dram_tensor("out", [LB, Z, A, K, NCLS], F32, kind="ExternalOutput")
    x_ap, o_ap = x_h.ap(), o_h.ap()
    if pe_shift:
        shm_h = nc.dram_tensor("shm", [P, P], F32, kind="ExternalInput")
        sel_h = nc.dram_tensor("sel", [P, P], F32, kind="ExternalInput")
    # bench mode: reps > 1 re-runs the whole kernel; each non-final pass
    # stores to its own DRAM scratch so stores are real traffic
    scratch_aps = [
        nc.dram_tensor(f"scr{r}", [LB, Z, A, K, NCLS], F32).ap()
        for r in range(reps - 1)]

    from contextlib import ExitStack
    with tile.TileContext(nc) as tc, ExitStack() as es:
        if pe_shift:
            consts = es.enter_context(tc.tile_pool(name="consts", bufs=1))
            psum = es.enter_context(
                tc.tile_pool(name="psum", bufs=2, space="PSUM"))
        with tc.tile_pool(name="pool", bufs=bufs) as pool:
            if pe_shift:
                sh_t = consts.tile([P, P], F32, name="sh_t")
                nc.sync.dma_start(sh_t[:], shm_h.ap()[:])
                sel_t = consts.tile([P, P], F32, name="sel_t")
                nc.sync.dma_start(sel_t[:], sel_h.ap()[:])
            for ci in range(NCH * reps):
                a0 = (ci % NCH) * ac
                lo, hi = max(0, a0 - 4), min(A, a0 + ac + 4)
                c_lo = (lo - (a0 - 4)) * C          # first valid xt col
                c_hi = (hi - (a0 - 4)) * C

                # ---- load x window (zero halo at image borders) ----
                # (b, z) rows are contiguous in DRAM: one 128-partition DMA
                xt = pool.tile([P, XW * C], F32, name="xt")
                if c_lo > 0:
                    nc.gpsimd.memset(_ap(xt, 0, P, 0, [[1, c_lo]]), 0.0)
                if c_hi < XW * C:
                    nc.gpsimd.memset(
                        _ap(xt, 0, P, c_hi, [[1, XW * C - c_hi]]), 0.0)
                nc.sync.dma_start(
                    _ap(xt, 0, P, c_lo, [[C, hi - lo], [1, C]]),
                    bass.AP(tensor=x_ap.tensor, offset=lo * C,
                            ap=[[X_Z, P], [C, hi - lo], [1, C]]))

                # ---- x_dn[p] = x at (z-1) (zeros at z=0 rows) ----
                # one DMA for rows 1..127 (row 64 gets cross-batch garbage,
                # re-zeroed after), plus halo/row-0 zeros from the memset
                x_dn = pool.tile([P, XW * C], F32, name="x_dn")
                nc.gpsimd.memset(x_dn[:], 0.0)
                for b in range(LB):
                    nc.sync.dma_start(
                        _ap(x_dn, b * Z + 1, Z - 1, c_lo, [[C, hi - lo], [1, C]]),
                        bass.AP(tensor=x_ap.tensor, offset=b * X_B + lo * C,
                                ap=[[X_Z, Z - 1], [C, hi - lo], [1, C]]))

                # ---- s = sum_c x^2 over the full x window ----
                sqx = pool.tile([P, XW * C], F32, name="sqx")
                nc.scalar.square(sqx[:], xt[:])
                st = pool.tile([P, XW], F32, name="st")
                nc.vector.tensor_reduce(
                    st[:], _ap(sqx, 0, P, 0, [[C, XW], [1, C]]),
                    axis=mybir.AxisListType.X, op=mybir.AluOpType.add)

                # ---- m_k maps: M[p, k*MW + ar] over a-window [a0-2, a0+258)
                # k=0..4: dz=-1, da=k-2 ; k=5,6: dz=0, da=k-7
                # d2 = (x - x_nbr)^2 in one fused custom op per k
                sqdiff = _get_sqdiff()
                M = pool.tile([P, 7 * MW], F32, name="M")
                dt5 = pool.tile([P, 5 * MW * C], F32, name="dt5")
                for k in range(5):
                    nc.vector._custom_dve(
                        sqdiff,
                        out=_ap(dt5, 0, P, k * MW * C, [[C, MW], [1, C]]),
                        in0=_ap(xt, 0, P, 2 * C, [[C, MW], [1, C]]),
                        in1=_ap(x_dn, 0, P, k * C, [[C, MW], [1, C]]))
                nc.vector.tensor_reduce(
                    _ap(M, 0, P, 0, [[1, 5 * MW]]),
                    _ap(dt5, 0, P, 0, [[C, 5 * MW], [1, C]]),
                    axis=mybir.AxisListType.X, op=mybir.AluOpType.add)

                dt2 = pool.tile([P, 2 * MW * C], F32, name="dt2")
                for k in (5, 6):
                    nc.vector._custom_dve(
                        sqdiff,
                        out=_ap(dt2, 0, P, (k - 5) * MW * C, [[C, MW], [1, C]]),
                        in0=_ap(xt, 0, P, 2 * C, [[C, MW], [1, C]]),
                        in1=_ap(xt, 0, P, (k - 5) * C, [[C, MW], [1, C]]))
                nc.vector.tensor_reduce(
                    _ap(M, 0, P, 5 * MW, [[1, 2 * MW]]),
                    _ap(dt2, 0, P, 0, [[C, 2 * MW], [1, C]]),
                    axis=mybir.AxisListType.X, op=mybir.AluOpType.add)

                # ---- M_up[p] = M[p+1] for k=0..4 cols; phantom z=64 rows
                # ({63,127}) = s(z=63 row) with k-dependent a-shift ----
                if pe_shift:
                    # PE permutation matmul: M_up = SH2^T.T @ M + SEL.T @ SD
                    # (exact for 0/1 matrices); phantom rows ride the second
                    # accumulating matmul through SD
                    SD = pool.tile([P, 5 * MW], F32, name="SD")
                    nc.vector.tensor_copy(
                        _ap(SD, 0, P, 0, [[MW, 5], [1, MW]]),
                        _ap(st, 0, P, 0, [[1, 5], [1, MW]]))
                    M_up = psum.tile([P, 5 * MW], F32, name="M_up_ps")
                    for n0 in range(0, 5 * MW, 512):
                        n1 = min(5 * MW, n0 + 512)
                        nc.tensor.matmul(
                            _ap(M_up, 0, P, n0, [[1, n1 - n0]]),
                            sh_t[:], _ap(M, 0, P, n0, [[1, n1 - n0]]),
                            start=True, stop=False)
                        nc.tensor.matmul(
                            _ap(M_up, 0, P, n0, [[1, n1 - n0]]),
                            sel_t[:], _ap(SD, 0, P, n0, [[1, n1 - n0]]),
                            start=False, stop=True)
                else:
                    M_up = pool.tile([P, 5 * MW], F32, name="M_up")
                    # disjoint remaps per batch so the phantom DMA runs parallel
                    for b in range(LB):
                        nc.sync.dma_start(
                            _ap(M_up, b * Z, Z - 1, 0, [[1, 5 * MW]]),
                            _ap(M, b * Z + 1, Z - 1, 0, [[1, 5 * MW]]))
                    # phantom: M_up[{63,127}, k*MW + ar] = st[{63,127}, ar + k]
                    nc.sync.dma_start(
                        _ap(M_up, Z - 1, 2, 0, [[MW, 5], [1, MW]], pstep=Z),
                        _ap(st, Z - 1, 2, 0, [[1, 5], [1, MW]], pstep=Z))

                # ---- exps into O staging [p, ar*56 + k*4 + c] ----
                O = pool.tile([P, ac * O_A], F32, name="O",
                              bufs=(1 if ac >= 512 else None))
                for th, sc in ((0, SC0), (1, SC1)):
                    co = 2 * th
                    # direct k=0..6: in M[p, k*MW + ar + 2]
                    nc.scalar.activation(
                        _ap(O, 0, P, co, [[4, 7], [O_A, ac], [1, 2]]),
                        _ap(M, 0, P, 2, [[MW, 7], [1, ac], [0, 2]]),
                        mybir.ActivationFunctionType.Exp, scale=sc)
                    # a-mirrors k'=7,8 <- k=6,5: col = k*MW + ar + (9-k)
                    nc.scalar.activation(
                        _ap(O, 0, P, 28 + co, [[4, 2], [O_A, ac], [1, 2]]),
                        _ap(M, 0, P, 6 * MW + 3, [[-(MW - 1), 2], [1, ac], [0, 2]]),
                        mybir.ActivationFunctionType.Exp, scale=sc)
                    # dz-mirrors k'=9..13 <- k=4..0: M_up[p, k*MW + ar + 4 - k]
                    nc.scalar.activation(
                        _ap(O, 0, P, 36 + co, [[4, 5], [O_A, ac], [1, 2]]),
                        _ap(M_up, 0, P, 4 * (MW - 1) + 4,
                            [[-(MW - 1), 5], [1, ac], [0, 2]]),
                        mybir.ActivationFunctionType.Exp, scale=sc)

                # ---- store: one contiguous 128-partition DMA ----
                rep_i = ci // NCH
                dst_ap = o_ap if rep_i == reps - 1 else scratch_aps[rep_i]
                nc.sync.dma_start(
                    bass.AP(tensor=dst_ap.tensor, offset=a0 * O_A,
                            ap=[[O_Z, P], [1, ac * O_A]]),
                    _ap(O, 0, P, 0, [[1, ac * O_A]]))

    nc.compile()
    return nc


class _Runner:
    """Compile once; reuse the jitted sharded executable across calls.

    Mirrors bass2jax.run_bass_via_pjrt's multi-core path, but without
    donated output buffers (the kernel writes every output element, so the
    zero "output operands" are passed once from device-resident buffers and
    reused)."""

    def __init__(self):
        import jax
        from jax.sharding import Mesh, PartitionSpec, NamedSharding
        try:
            from jax.experimental.shard_map import shard_map
        except ImportError:
            from jax.shard_map import shard_map  # newer jax
        from concourse import bass2jax

        bass2jax.install_neuronx_cc_hook()
        nc = _build()
        self.nc = nc

        partition_name = (nc.partition_id_tensor.name
                          if nc.partition_id_tensor else None)
        in_names, out_names, out_avals = [], [], []
        for alloc in nc.m.functions[0].allocations:
            if not isinstance(alloc, mybir.MemoryLocationSet):
                continue
            name = alloc.memorylocations[0].name
            if alloc.kind == "ExternalInput":
                if name != partition_name:
                    in_names.append(name)
            elif alloc.kind == "ExternalOutput":
                out_names.append(name)
                out_avals.append(jax.core.ShapedArray(
                    tuple(alloc.tensor_shape), mybir.dt.np(alloc.dtype)))
        assert set(in_names) <= {"x", "shm", "sel"}, in_names
        assert out_names == ["out"], out_names
        all_in_names = in_names + out_names
        if partition_name is not None:
            all_in_names = all_in_names + [partition_name]
        self.in_names = in_names

        def _body(*args):
            operands = list(args)
            if partition_name is not None:
                operands.append(bass2jax.partition_id_tensor())
            return tuple(bass2jax._bass_exec_p.bind(
                *operands,
                out_avals=tuple(out_avals),
                in_names=tuple(all_in_names),
                out_names=tuple(out_names),
                lowering_input_output_aliases=(),
                sim_require_finite=True,
                sim_require_nnan=True,
                nc=nc,
            ))

        devices = jax.devices()[:N_CORES]
        assert len(devices) == N_CORES
        self.mesh = Mesh(np.asarray(devices), ("core",))
        spec = PartitionSpec("core")
        rep = PartitionSpec()
        self.sharding = NamedSharding(self.mesh, spec)
        in_specs = tuple(spec if n == "x" else rep for n in in_names) + (spec,)
        self.jitted = jax.jit(shard_map(
            _body, mesh=self.mesh, in_specs=in_specs, out_specs=(spec,),
            check_rep=False))
        # device-resident constant operands, created once
        self.zeros_dev = jax.device_put(
            np.zeros((N_CORES * LB, Z, A, K, NCLS), np.float32), self.sharding)
        consts = {}
        if "shm" in in_names or "sel" in in_names:
            shm, sel = _host_shift_mats()
            rep_sh = NamedSharding(self.mesh, rep)
            consts["shm"] = jax.device_put(shm, rep_sh)
            consts["sel"] = jax.device_put(sel, rep_sh)
        self.consts = consts
        self._jax = jax

    def put(self, x: np.ndarray):
        return self._jax.device_put(
            np.ascontiguousarray(np.asarray(x, np.float32)), self.sharding)

    def run_dev(self, x_dev):
        """Execute; returns device array (not fetched)."""
        args = [x_dev if n == "x" else self.consts[n] for n in self.in_names]
        return self.jitted(*args, self.zeros_dev)[0]

    def __call__(self, x: np.ndarray) -> np.ndarray:
        return np.asarray(self.run_dev(self.put(x)))


_RUNNER = None


def _get_runner():
    global _RUNNER
    if _RUNNER is None:
        _RUNNER = _Runner()
    return _RUNNER


def kernel(x: np.ndarray) -> np.ndarray:
    x = np.asarray(x, dtype=np.float32)
    assert x.shape == (B, Z, A, C), x.shape
    try:
        return _get_runner()(x)
    except Exception:
        # fallback: reference-quality but slower dispatch path
        nc = _build()
        extra = {}
        if PE_SHIFT:
            shm, sel = _host_shift_mats()
            extra = {"shm": shm, "sel": sel}
        in_maps = [{"x": np.ascontiguousarray(x[i * LB:(i + 1) * LB]), **extra}
                   for i in range(N_CORES)]
        res = run_bass_kernel_spmd(nc, in_maps, list(range(N_CORES)))
        return np.concatenate(
            [res.results[i]["out"] for i in range(N_CORES)], axis=0)


# revision 42
# speedup vs baseline: 6127.7813x; 1.7514x over previous
"""BilateralFilter (SqueezeSeg mc condensing-kernel gaussians) on 8 TRN2 cores.

Reference computes, for x: [16, 64, 512, 3] (B, Z, A, C=xyz):
    nbr   = 14 spatial neighbors of each pixel in a 3x5 window (zero-padded)
    diff2 = sum_c (x - nbr)^2                           [B, Z, A, 14]
    out   = exp(-diff2 / (2 * theta_r^2))               [B, Z, A, 14, 4]
with THETA_R = [0.015, 0.015, 0.01, 0.01] (only 2 distinct values).

Strategy (pure batch data-parallel, 2 batches per core):
  - partitions p = b*64 + z  (128), free dim = azimuth chunks (AC wide).
  - squared differences via a runtime-registered fused custom DVE op
    (out = (in0-in1)^2), channel sums via tensor_reduce.
  - mirror symmetry: m_k(q) = |x(q) - x(q+off_k)|^2 for the 7 "negative"
    offsets k=0..6 gives the other 7 via diff2_{13-k}(q) = m_k(q - off_k);
    the z+1-partition read (engines cannot shift partitions by 1) is
    materialized on the idle TensorE as an exact 0/1 permutation matmul
    into PSUM, with the phantom z=64 boundary row (out-of-image neighbor
    => diff2 = |x(center)|^2, from s = sum_c x^2) accumulated by a second
    selector matmul. (PE_SHIFT=False falls back to partition-remap DMAs.)
  - ACT computes exp with the free scale immediate; each exp is written to
    both classes of its theta pair via a stride-0 input axis, directly into
    the interleaved [a, k, c] staging layout.
  - the staging tile matches DRAM layout exactly, so the store is one
    contiguous 128-partition DMA (28 KB/partition runs at AC=128).
"""

import numpy as np

import concourse.bass as bass
import concourse.tile as tile
from concourse import bacc, mybir
from concourse.bass_utils import run_bass_kernel_spmd

N_CORES = 8
B, Z, A, C = 16, 64, 512, 3
K, NCLS = 14, 4
LB = B // N_CORES            # local batches per core = 2
P = LB * Z                   # 128 partitions
AC = 128                     # azimuth chunk
BUFS = 3                     # tile pool buffers
PE_SHIFT = True              # z+1 partition shift via PE matmul vs SBUF DMA
XDN_PE = False               # derive x_dn on PE too (serializes behind xt load)
F32 = mybir.dt.float32


def _host_shift_mats():
    """SH2[k, m] = 1 iff k == m+1 (and not m == 63: batch boundary);
    SEL[k, m] = 1 iff k == m in {63, 127} (phantom z=64 row selector);
    SHD[k, m] = 1 iff k == m-1 (and not m in {0, 64}: z=0 rows stay 0)."""
    sh = np.zeros((P, P), np.float32)
    for m in range(P - 1):
        if m != Z - 1:
            sh[m + 1, m] = 1.0
    sel = np.zeros((P, P), np.float32)
    sel[Z - 1, Z - 1] = 1.0
    sel[P - 1, P - 1] = 1.0
    shd = np.zeros((P, P), np.float32)
    for m in range(1, P):
        if m != Z:
            shd[m - 1, m] = 1.0
    return sh, sel, shd

# exp scales: -1 / (2 * theta^2), theta pairs (0.015, 0.01), f32 semantics
_t0 = np.float32(0.015)
_t1 = np.float32(0.01)
SC0 = -float(1.0 / np.float32(np.float32(2.0) * _t0 * _t0))
SC1 = -float(1.0 / np.float32(np.float32(2.0) * _t1 * _t1))

# DRAM strides (elements) of out [LB, Z, A, K, NCLS]
O_A = K * NCLS               # 56
O_Z = A * O_A                # 28672
O_B = Z * O_Z                # 1835008
X_Z = A * C                  # 1536
X_B = Z * X_Z


def _ap(t, poff, pcnt, foff, pairs, pstep=1):
    """AP on tile t: partitions [poff, poff+pcnt) (stride pstep rows), free
    `pairs` ([step, count] in elements) based at element foff."""
    row = t.ap[0][0]
    return bass.AP(tensor=t.tensor, offset=t.offset + poff * row + foff,
                   ap=[[pstep * row, pcnt]] + [list(p) for p in pairs])


_SQDIFF = None


def _get_sqdiff():
    """Register a runtime custom DVE op: out = (in0 - in1)^2 (fp32, one
    instruction instead of subtract + multiply)."""
    global _SQDIFF
    if _SQDIFF is not None:
        return _SQDIFF
    from concourse import dve_ops
    from concourse.dve_spec import Spec, Src0, Src1, sq, lower, _has_src1
    from concourse.dve_uop import DveOpSpec

    name = "SQDIFF_BILAT_ANT"
    if name not in dve_ops._SUB_OPCODE_FOR_NAME:
        spec = Spec(
            body=sq(Src0 - Src1),
            reference=lambda in0, in1, c0, c1, c2:
                (in0.astype(np.float32) - in1.astype(np.float32)) ** 2)
        row = 1 + len(dve_ops.OPS)
        assert row < 0x20
        shas = {}
        for ver in ("v3",):
            tmp = DveOpSpec(name=name, opcode=row, uops=lower(spec, ver=ver),
                            rd1_en=_has_src1(spec))
            shas[ver] = tmp.sha(ver)
        op = dve_ops.DveOp(name, spec, subdim=False, uops_sha=shas)
        dve_ops.OPS.append(op)
        dve_ops.CUSTOM_DVE_SPECS[name] = spec
        dve_ops._SUB_OPCODE_FOR_NAME[name] = row
    else:
        op = next(o for o in dve_ops.OPS if o.name == name)
    _SQDIFF = op
    return op


def _build(ac=AC, bufs=BUFS, reps=1, pe_shift=PE_SHIFT, xdn_pe=XDN_PE):
    XW = ac + 8                  # x window (halo 4 each side)
    MW = ac + 4                  # m window (halo 2 each side)
    NCH = A // ac
    nc = bacc.Bacc("TRN2", target_bir_lowering=False, debug=False,
                   num_devices=N_CORES)
    x_h = nc.dram_tensor("x", [LB, Z, A, C], F32, kind="ExternalInput")
    o_h = nc.dram_tensor("out", [LB, Z, A, K, NCLS], F32, kind="ExternalOutput")
    x_ap, o_ap = x_h.ap(), o_h.ap()
    if pe_shift:
        shm_h = nc.dram_tensor("shm", [P, P], F32, kind="ExternalInput")
        sel_h = nc.dram_tensor("sel", [P, P], F32, kind="ExternalInput")
        if xdn_pe:
            shd_h = nc.dram_tensor("shd", [P, P], F32, kind="ExternalInput")
    # bench mode: reps > 1 re-runs the whole kernel; each non-final pass
    # stores to its own DRAM scratch so stores are real traffic
    scratch_aps = [
        nc.dram_tensor(f"scr{r}", [LB, Z, A, K, NCLS], F32).ap()
        for r in range(reps - 1)]

    from contextlib import ExitStack
    with tile.TileContext(nc) as tc, ExitStack() as es:
        if pe_shift:
            consts = es.enter_context(tc.tile_pool(name="consts", bufs=1))
            psum = es.enter_context(
                tc.tile_pool(name="psum", bufs=2, space="PSUM"))
        with tc.tile_pool(name="pool", bufs=bufs) as pool:
            if pe_shift:
                sh_t = consts.tile([P, P], F32, name="sh_t")
                nc.sync.dma_start(sh_t[:], shm_h.ap()[:])
                sel_t = consts.tile([P, P], F32, name="sel_t")
                nc.sync.dma_start(sel_t[:], sel_h.ap()[:])
                if xdn_pe:
                    shd_t = consts.tile([P, P], F32, name="shd_t")
                    nc.sync.dma_start(shd_t[:], shd_h.ap()[:])
            for ci in range(NCH * reps):
                a0 = (ci % NCH) * ac
                lo, hi = max(0, a0 - 4), min(A, a0 + ac + 4)
                c_lo = (lo - (a0 - 4)) * C          # first valid xt col
                c_hi = (hi - (a0 - 4)) * C

                # ---- load x window (zero halo at image borders) ----
                # (b, z) rows are contiguous in DRAM: one 128-partition DMA
                xt = pool.tile([P, XW * C], F32, name="xt")
                if c_lo > 0:
                    nc.gpsimd.memset(_ap(xt, 0, P, 0, [[1, c_lo]]), 0.0)
                if c_hi < XW * C:
                    nc.gpsimd.memset(
                        _ap(xt, 0, P, c_hi, [[1, XW * C - c_hi]]), 0.0)
                nc.sync.dma_start(
                    _ap(xt, 0, P, c_lo, [[C, hi - lo], [1, C]]),
                    bass.AP(tensor=x_ap.tensor, offset=lo * C,
                            ap=[[X_Z, P], [C, hi - lo], [1, C]]))

                # ---- x_dn[p] = x at (z-1) (zeros at z=0 rows) ----
                if pe_shift and xdn_pe:
                    # exact PE permutation shift of xt into PSUM; the zero
                    # columns of SHD give the z=0 rows (and the xt halo gives
                    # the image-border zeros) for free
                    x_dn = psum.tile([P, XW * C], F32, name="x_dn_ps")
                    for n0 in range(0, XW * C, 512):
                        n1 = min(XW * C, n0 + 512)
                        nc.tensor.matmul(
                            _ap(x_dn, 0, P, n0, [[1, n1 - n0]]),
                            shd_t[:], _ap(xt, 0, P, n0, [[1, n1 - n0]]),
                            start=True, stop=True)
                else:
                    x_dn = pool.tile([P, XW * C], F32, name="x_dn")
                    nc.gpsimd.memset(x_dn[:], 0.0)
                    for b in range(LB):
                        nc.sync.dma_start(
                            _ap(x_dn, b * Z + 1, Z - 1, c_lo,
                                [[C, hi - lo], [1, C]]),
                            bass.AP(tensor=x_ap.tensor, offset=b * X_B + lo * C,
                                    ap=[[X_Z, Z - 1], [C, hi - lo], [1, C]]))

                # ---- s = sum_c x^2 over the full x window ----
                sqx = pool.tile([P, XW * C], F32, name="sqx")
                nc.scalar.square(sqx[:], xt[:])
                st = pool.tile([P, XW], F32, name="st")
                nc.vector.tensor_reduce(
                    st[:], _ap(sqx, 0, P, 0, [[C, XW], [1, C]]),
                    axis=mybir.AxisListType.X, op=mybir.AluOpType.add)

                # ---- m_k maps: M[p, k*MW + ar] over a-window [a0-2, a0+258)
                # k=0..4: dz=-1, da=k-2 ; k=5,6: dz=0, da=k-7
                # d2 = (x - x_nbr)^2 in one fused custom op per k
                sqdiff = _get_sqdiff()
                M = pool.tile([P, 7 * MW], F32, name="M")
                dt5 = pool.tile([P, 5 * MW * C], F32, name="dt5")
                for k in range(5):
                    nc.vector._custom_dve(
                        sqdiff,
                        out=_ap(dt5, 0, P, k * MW * C, [[C, MW], [1, C]]),
                        in0=_ap(xt, 0, P, 2 * C, [[C, MW], [1, C]]),
                        in1=_ap(x_dn, 0, P, k * C, [[C, MW], [1, C]]))
                nc.vector.tensor_reduce(
                    _ap(M, 0, P, 0, [[1, 5 * MW]]),
                    _ap(dt5, 0, P, 0, [[C, 5 * MW], [1, C]]),
                    axis=mybir.AxisListType.X, op=mybir.AluOpType.add)

                dt2 = pool.tile([P, 2 * MW * C], F32, name="dt2")
                for k in (5, 6):
                    nc.vector._custom_dve(
                        sqdiff,
                        out=_ap(dt2, 0, P, (k - 5) * MW * C, [[C, MW], [1, C]]),
                        in0=_ap(xt, 0, P, 2 * C, [[C, MW], [1, C]]),
                        in1=_ap(xt, 0, P, (k - 5) * C, [[C, MW], [1, C]]))
                nc.vector.tensor_reduce(
                    _ap(M, 0, P, 5 * MW, [[1, 2 * MW]]),
                    _ap(dt2, 0, P, 0, [[C, 2 * MW], [1, C]]),
                    axis=mybir.AxisListType.X, op=mybir.AluOpType.add)

                # ---- M_up[p] = M[p+1] for k=0..4 cols; phantom z=64 rows
                # ({63,127}) = s(z=63 row) with k-dependent a-shift ----
                if pe_shift:
                    # PE permutation matmul: M_up = SH2^T.T @ M + SEL.T @ SD
                    # (exact for 0/1 matrices); phantom rows ride the second
                    # accumulating matmul through SD
                    SD = pool.tile([P, 5 * MW], F32, name="SD")
                    nc.vector.tensor_copy(
                        _ap(SD, 0, P, 0, [[MW, 5], [1, MW]]),
                        _ap(st, 0, P, 0, [[1, 5], [1, MW]]))
                    M_up = psum.tile([P, 5 * MW], F32, name="M_up_ps")
                    for n0 in range(0, 5 * MW, 512):
                        n1 = min(5 * MW, n0 + 512)
                        nc.tensor.matmul(
                            _ap(M_up, 0, P, n0, [[1, n1 - n0]]),
                            sh_t[:], _ap(M, 0, P, n0, [[1, n1 - n0]]),
                            start=True, stop=False)
                        nc.tensor.matmul(
                            _ap(M_up, 0, P, n0, [[1, n1 - n0]]),
                            sel_t[:], _ap(SD, 0, P, n0, [[1, n1 - n0]]),
                            start=False, stop=True)
                else:
                    M_up = pool.tile([P, 5 * MW], F32, name="M_up")
                    # disjoint remaps per batch so the phantom DMA runs parallel
                    for b in range(LB):
                        nc.sync.dma_start(
                            _ap(M_up, b * Z, Z - 1, 0, [[1, 5 * MW]]),
                            _ap(M, b * Z + 1, Z - 1, 0, [[1, 5 * MW]]))
                    # phantom: M_up[{63,127}, k*MW + ar] = st[{63,127}, ar + k]
                    nc.sync.dma_start(
                        _ap(M_up, Z - 1, 2, 0, [[MW, 5], [1, MW]], pstep=Z),
                        _ap(st, Z - 1, 2, 0, [[1, 5], [1, MW]], pstep=Z))

                # ---- exps into O staging [p, ar*56 + k*4 + c] ----
                O = pool.tile([P, ac * O_A], F32, name="O",
                              bufs=(1 if ac >= 512 else None))
                for th, sc in ((0, SC0), (1, SC1)):
                    co = 2 * th
                    # direct k=0..6: in M[p, k*MW + ar + 2]
                    nc.scalar.activation(
                        _ap(O, 0, P, co, [[4, 7], [O_A, ac], [1, 2]]),
                        _ap(M, 0, P, 2, [[MW, 7], [1, ac], [0, 2]]),
                        mybir.ActivationFunctionType.Exp, scale=sc)
                    # a-mirrors k'=7,8 <- k=6,5: col = k*MW + ar + (9-k)
                    nc.scalar.activation(
                        _ap(O, 0, P, 28 + co, [[4, 2], [O_A, ac], [1, 2]]),
                        _ap(M, 0, P, 6 * MW + 3, [[-(MW - 1), 2], [1, ac], [0, 2]]),
                        mybir.ActivationFunctionType.Exp, scale=sc)
                    # dz-mirrors k'=9..13 <- k=4..0: M_up[p, k*MW + ar + 4 - k]
                    nc.scalar.activation(
                        _ap(O, 0, P, 36 + co, [[4, 5], [O_A, ac], [1, 2]]),
                        _ap(M_up, 0, P, 4 * (MW - 1) + 4,
                            [[-(MW - 1), 5], [1, ac], [0, 2]]),
                        mybir.ActivationFunctionType.Exp, scale=sc)

                # ---- store: one contiguous 128-partition DMA ----
                rep_i = ci // NCH
                dst_ap = o_ap if rep_i == reps - 1 else scratch_aps[rep_i]
                nc.sync.dma_start(
                    bass.AP(tensor=dst_ap.tensor, offset=a0 * O_A,
                            ap=[[O_Z, P], [1, ac * O_A]]),
                    _ap(O, 0, P, 0, [[1, ac * O_A]]))

    nc.compile()
    return nc


class _Runner:
    """Compile once; reuse the jitted sharded executable across calls.

    Mirrors bass2jax.run_bass_via_pjrt's multi-core path, but without
    donated output buffers (the kernel writes every output element, so the
    zero "output operands" are passed once from device-resident buffers and
    reused)."""

    def __init__(self):
        import jax
        from jax.sharding import Mesh, PartitionSpec, NamedSharding
        try:
            from jax.experimental.shard_map import shard_map
        except ImportError:
            from jax.shard_map import shard_map  # newer jax
        from concourse import bass2jax

        bass2jax.install_neuronx_cc_hook()
        nc = _build()
        self.nc = nc

        partition_name = (nc.partition_id_tensor.name
                          if nc.partition_id_tensor else None)
        in_names, out_names, out_avals = [], [], []
        for alloc in nc.m.functions[0].allocations:
            if not isinstance(alloc, mybir.MemoryLocationSet):
                continue
            name = alloc.memorylocations[0].name
            if alloc.kind == "ExternalInput":
                if name != partition_name:
                    in_names.append(name)
            elif alloc.kind == "ExternalOutput":
                out_names.append(name)
                out_avals.append(jax.core.ShapedArray(
                    tuple(alloc.tensor_shape), mybir.dt.np(alloc.dtype)))
        assert set(in_names) <= {"x", "shm", "sel", "shd"}, in_names
        assert out_names == ["out"], out_names
        all_in_names = in_names + out_names
        if partition_name is not None:
            all_in_names = all_in_names + [partition_name]
        self.in_names = in_names

        def _body(*args):
            operands = list(args)
            if partition_name is not None:
                operands.append(bass2jax.partition_id_tensor())
            return tuple(bass2jax._bass_exec_p.bind(
                *operands,
                out_avals=tuple(out_avals),
                in_names=tuple(all_in_names),
                out_names=tuple(out_names),
                lowering_input_output_aliases=(),
                sim_require_finite=True,
                sim_require_nnan=True,
                nc=nc,
            ))

        devices = jax.devices()[:N_CORES]
        assert len(devices) == N_CORES
        self.mesh = Mesh(np.asarray(devices), ("core",))
        spec = PartitionSpec("core")
        rep = PartitionSpec()
        self.sharding = NamedSharding(self.mesh, spec)
        in_specs = tuple(spec if n == "x" else rep for n in in_names) + (spec,)
        self.jitted = jax.jit(shard_map(
            _body, mesh=self.mesh, in_specs=in_specs, out_specs=(spec,),
            check_rep=False))
        # device-resident constant operands, created once
        self.zeros_dev = jax.device_put(
            np.zeros((N_CORES * LB, Z, A, K, NCLS), np.float32), self.sharding)
        consts = {}
        if "shm" in in_names:
            shm, sel, shd = _host_shift_mats()
            rep_sh = NamedSharding(self.mesh, rep)
            consts["shm"] = jax.device_put(shm, rep_sh)
            consts["sel"] = jax.device_put(sel, rep_sh)
            consts["shd"] = jax.device_put(shd, rep_sh)
        self.consts = consts
        self._jax = jax

    def put(self, x: np.ndarray):
        return self._jax.device_put(
            np.ascontiguousarray(np.asarray(x, np.float32)), self.sharding)

    def run_dev(self, x_dev):
        """Execute; returns device array (not fetched)."""
        args = [x_dev if n == "x" else self.consts[n] for n in self.in_names]
        return self.jitted(*args, self.zeros_dev)[0]

    def __call__(self, x: np.ndarray) -> np.ndarray:
        return np.asarray(self.run_dev(self.put(x)))


_RUNNER = None


def _get_runner():
    global _RUNNER
    if _RUNNER is None:
        _RUNNER = _Runner()
    return _RUNNER


def kernel(x: np.ndarray) -> np.ndarray:
    x = np.asarray(x, dtype=np.float32)
    assert x.shape == (B, Z, A, C), x.shape
    try:
        return _get_runner()(x)
    except Exception:
        # fallback: reference-quality but slower dispatch path
        nc = _build()
        extra = {}
        if PE_SHIFT:
            shm, sel, shd = _host_shift_mats()
            extra = {"shm": shm, "sel": sel, "shd": shd}
        in_maps = [{"x": np.ascontiguousarray(x[i * LB:(i + 1) * LB]), **extra}
                   for i in range(N_CORES)]
        res = run_bass_kernel_spmd(nc, in_maps, list(range(N_CORES)))
        return np.concatenate(
            [res.results[i]["out"] for i in range(N_CORES)], axis=0)
